# revision 2
# baseline (speedup 1.0000x reference)
"""Trainium2 Bass kernel for nn_Attention_4415226380830 (XCA channel attention),
restructured around fp8 DoubleRow matmuls.

Dtype strategy (validated vs reference in numpy):
  - q,k path: fully fp8e4m3 (errors wash out in the 65536-px Gram).
  - v path: x and conv weights split hi+lo fp8 (3-term products ~= fp16
    accuracy at 2x fp16 matmul rate); dw for v-chunk3 likewise hi/lo DR
    pairs on PE; v-chunk4 (64 ch) taps on DVE/Pool at fp16.
  - v spill + out: fp16.

PE work per macro (16 rows): conv qk 3xDR + v 2x3xDR, dw as DR *tap pairs*
(2 shifted windows per matmul via a stride-delta kt dim), packed fp16-view
transposes (2 px/row), DR gram (256 px per pass). Norms via ACT Square+accum
on fp8 acc. Evacs split ACT/DVE (Pool cannot read PSUM).
"""

import numpy as np
from contextlib import ExitStack

import ml_dtypes
import concourse.bass as bass
from concourse import bacc
import concourse.mybir as mybir
import concourse.tile as tile
from concourse.bass_utils import run_bass_kernel_spmd

F32 = mybir.dt.float32
F16 = mybir.dt.float16
F8 = mybir.dt.float8e4
NP8 = ml_dtypes.float8_e4m3fn
DR = mybir.MatmulPerfMode.DoubleRow

B = 4
C = 192
HEADS = 4
DH = C // HEADS
H = 256
W = 256
C3 = 3 * C
N_CORES = 8
EPS = 1e-12

ROWS = H // 2            # rows per core
MROWS = 16               # rows per macro-tile
NMACRO = ROWS // MROWS   # 8
WIN_ROWS = MROWS + 2     # 18
PXM = MROWS * W          # 4096

# conv psum substeps over the 18-row window: 4+4+4+4+2 rows
CONV_STEPS = [(r, min(4, WIN_ROWS - r)) for r in range(0, WIN_ROWS, 4)]
# dw substeps: 4 rows = 1024 px each
DWSTEPS = 4
DWROWS = MROWS // DWSTEPS  # 4 rows per substep
# tap pairs (t = 3*(dy+1) + (dx+1)); last pair duplicates tap 8 with zero B
TAP_PAIRS = [(0, 1), (2, 3), (4, 5), (6, 7), (8, 8)]


def _tap_delta(tA, tB):
    dyA, dxA = tA // 3 - 1, tA % 3 - 1
    dyB, dxB = tB // 3 - 1, tB % 3 - 1
    return (dyB - dyA) * 258 + (dxB - dxA)


def _dw_rhs(win, cp, si2, tA, tB):
    """DR moving AP: pair of shifted 2-row windows from win [cp,18,258]."""
    dyA, dxA = tA // 3 - 1, tA % 3 - 1
    base = win[0:cp, 1 + dyA + 2 * si2:1 + dyA + 2 * si2 + 2,
               1 + dxA:257 + dxA]
    return bass.AP(tensor=win.tensor, offset=base.offset,
                   ap=[[win.ap[0][0], cp], [_tap_delta(tA, tB), 2],
                       [258, 2], [1, 256]])


def _build_phase1():
    nc = bacc.Bacc("TRN2", target_bir_lowering=False, debug=False,
                   num_devices=N_CORES)
    # x hi/lo, kt-padded to 256 channels (192:256 zeroed on host)
    x_hl = nc.dram_tensor("x_hl", [2, 256, ROWS + 2, W], F8,
                          kind="ExternalInput").ap()
    wqk_in = nc.dram_tensor("wqk", [128, 3, 2, 128], F8, kind="ExternalInput").ap()
    wv_in = nc.dram_tensor("wv", [128, 2, 2, 2, 128], F8, kind="ExternalInput").ap()
    dq_in = nc.dram_tensor("dq", [128, 3, 5, 2, 128], F8, kind="ExternalInput").ap()
    dv_in = nc.dram_tensor("dv", [128, 2, 5, 2, 128], F8, kind="ExternalInput").ap()
    dv4_in = nc.dram_tensor("dv4", [64, 2, 5, 2, 64], F8, kind="ExternalInput").ap()
    ident_in = nc.dram_tensor("ident_in", [128, 128], F16, kind="ExternalInput").ap()

    gram_out = nc.dram_tensor("gram_part", [128, 384], F32, kind="ExternalOutput").ap()
    sumsq_out = nc.dram_tensor("sumsq_part", [128, 384], F32,
                               kind="ExternalOutput").ap()
    v_out = nc.dram_tensor("v_sp", [C, ROWS * W], F16, kind="ExternalOutput").ap()

    with ExitStack() as ctx:
        tc = ctx.enter_context(tile.TileContext(nc))
        consts = ctx.enter_context(tc.tile_pool(name="consts", bufs=1))
        xpool = ctx.enter_context(tc.tile_pool(name="xpool", bufs=2))
        winp = ctx.enter_context(tc.tile_pool(name="winp", bufs=4))
        winvp = ctx.enter_context(tc.tile_pool(name="winvp", bufs=2))
        accp = ctx.enter_context(tc.tile_pool(name="accp", bufs=4))
        vaccp = ctx.enter_context(tc.tile_pool(name="vaccp", bufs=2))
        qkTp = ctx.enter_context(tc.tile_pool(name="qkTp", bufs=4))
        ps_big = ctx.enter_context(tc.tile_pool(name="ps_big", bufs=4, space="PSUM"))
        ps_t = ctx.enter_context(tc.tile_pool(name="ps_t", bufs=2, space="PSUM"))
        ps_g = ctx.enter_context(tc.tile_pool(name="ps_g", bufs=1, space="PSUM"))
        ps_g2 = ctx.enter_context(tc.tile_pool(name="ps_g2", bufs=1, space="PSUM"))

        wqk = consts.tile([128, 3, 2, 128], F8, tag="wqk")
        wv = consts.tile([128, 2, 2, 2, 128], F8, tag="wv")
        dq = consts.tile([128, 3, 5, 2, 128], F8, tag="dq")
        dv = consts.tile([128, 2, 5, 2, 128], F8, tag="dv")
        dv4 = consts.tile([64, 2, 5, 2, 64], F8, tag="dv4")
        ident = consts.tile([128, 128], F16, tag="ident")
        nc.sync.dma_start(out=wqk, in_=wqk_in)
        nc.sync.dma_start(out=wv, in_=wv_in)
        nc.sync.dma_start(out=dq, in_=dq_in)
        nc.sync.dma_start(out=dv, in_=dv_in)
        nc.sync.dma_start(out=dv4, in_=dv4_in)
        nc.sync.dma_start(out=ident, in_=ident_in)

        gram_ps = ps_g.tile([128, 384], F32)
        gram2_ps = ps_g2.tile([128, 384], F32)

        for mj in range(NMACRO):
            r0 = MROWS * mj
            # x tile [128, kt, hl, 18, 256]
            xt = xpool.tile([128, 2, 2, WIN_ROWS, W], F8, tag="xt")
            for kt in range(2):
                for hl in range(2):
                    nc.sync.dma_start(
                        out=xt[:, kt, hl],
                        in_=x_hl[hl, 128 * kt:128 * (kt + 1), r0:r0 + WIN_ROWS, :])

            accs = {}
            wins = {}

            def conv_qk(ci):
                win = winp.tile([128, WIN_ROWS, 258], F8, tag="win")
                nc.gpsimd.memset(win[:, :, 0:258:257], 0.0)
                for st in range(WIN_ROWS // 2):
                    sr = 2 * st
                    pc = ps_big.tile([128, 512], F32, tag="pw")
                    nc.tensor.matmul(pc, wqk[:, ci],
                                     xt[:, :, 0, sr:sr + 2, :],
                                     start=True, stop=True, perf_mode=DR)
                    dst = win[:, sr:sr + 2, 1:257]
                    srcc = pc.rearrange("p (r w) -> p r w", w=W)
                    if st % 2 == 0:
                        nc.scalar.copy(out=dst, in_=srcc)
                    else:
                        nc.vector.tensor_copy(out=dst, in_=srcc)
                wins[ci] = win

            def conv_v(cj):
                cp = 128 if cj == 0 else 64
                wh = winvp.tile([cp, WIN_ROWS, 258], F8, tag=f"wh{cj}")
                wl = winvp.tile([cp, WIN_ROWS, 258], F8, tag=f"wl{cj}")
                nc.gpsimd.memset(wh[:, :, 0:258:257], 0.0)
                nc.gpsimd.memset(wl[:, :, 0:258:257], 0.0)
                for st in range(WIN_ROWS // 2):
                    sr = 2 * st
                    rs = slice(sr, sr + 2)
                    pc = ps_big.tile([128, 512], F32, tag="pw")
                    nc.tensor.matmul(pc[0:cp], wv[:, cj, 0, :, 0:cp],
                                     xt[:, :, 0, rs, :],
                                     start=True, stop=False, perf_mode=DR)
                    nc.tensor.matmul(pc[0:cp], wv[:, cj, 0, :, 0:cp],
                                     xt[:, :, 1, rs, :],
                                     start=False, stop=False, perf_mode=DR)
                    nc.tensor.matmul(pc[0:cp], wv[:, cj, 1, :, 0:cp],
                                     xt[:, :, 0, rs, :],
                                     start=False, stop=True, perf_mode=DR)
                    srcv = pc[0:cp].rearrange("p (r w) -> p r w", w=W)
                    nc.scalar.copy(out=wh[:, rs, 1:257], in_=srcv)
                    nc.vector.tensor_tensor(out=wl[:, rs, 1:257],
                                            in0=srcv, in1=wh[:, rs, 1:257],
                                            op=mybir.AluOpType.subtract)
                wins[3 + cj] = (wh, wl, cp)

            def dw_qk(ci):
                acc = accp.tile([128, PXM], F8, tag="acc")
                for si2 in range(2 * DWSTEPS):
                    pd = ps_big.tile([128, 512], F32, tag="pw")
                    for pi, (tA, tB) in enumerate(TAP_PAIRS):
                        nc.tensor.matmul(pd, dq[:, ci, pi],
                                         _dw_rhs(wins[ci], 128, si2, tA, tB),
                                         start=(pi == 0), stop=(pi == 4),
                                         perf_mode=DR)
                    if si2 % 2 == 0:
                        nc.scalar.copy(out=acc[:, si2 * 512:(si2 + 1) * 512], in_=pd)
                    else:
                        nc.vector.tensor_copy(out=acc[:, si2 * 512:(si2 + 1) * 512],
                                              in_=pd)
                accs[ci] = acc

            def dw_v(cj, vacc):
                wh, wl, cp = wins[3 + cj]
                dvt = dv if cj == 0 else dv4
                for si2 in range(2 * DWSTEPS):
                    pd = ps_big.tile([128, 512], F32, tag="pw")
                    k = 0
                    for (wset, wn) in ((0, wh), (0, wl), (1, wh)):
                        for pi, (tA, tB) in enumerate(TAP_PAIRS):
                            nc.tensor.matmul(pd[0:cp], dvt[:, wset, pi],
                                             _dw_rhs(wn, cp, si2, tA, tB),
                                             start=(k == 0), stop=(k == 14),
                                             perf_mode=DR)
                            k += 1
                    if si2 % 2 == 0:
                        nc.scalar.copy(out=vacc[:, si2 * 512:(si2 + 1) * 512],
                                       in_=pd[0:cp])
                    else:
                        nc.vector.tensor_copy(out=vacc[:, si2 * 512:(si2 + 1) * 512],
                                              in_=pd[0:cp])

            vacc3 = vaccp.tile([128, PXM], F16, tag="v3")
            vacc4 = vaccp.tile([64, PXM], F16, tag="v4")
            # software pipeline: conv(i+1) issues before dw(i)
            conv_qk(0)
            conv_qk(1)
            dw_qk(0)
            conv_qk(2)
            dw_qk(1)
            conv_v(0)
            dw_qk(2)
            conv_v(1)
            dw_v(0, vacc3)
            dw_v(1, vacc4)

            # ======== Phase C: spill v, transposes, grams ========
            nc.sync.dma_start(out=v_out[0:128, mj * PXM:(mj + 1) * PXM], in_=vacc3)
            nc.sync.dma_start(out=v_out[128:192, mj * PXM:(mj + 1) * PXM], in_=vacc4)

            a16 = [accs[ci].bitcast(F16) for ci in range(3)]   # [128, 2048]
            for pg in range(PXM // 512):    # 2 pr per group
                tps = ps_t.tile([128, 2, 384], F16, tag="tps")
                for hp in range(2):
                    for ci in range(3):
                        nc.tensor.matmul(
                            tps[:, hp, ci * 128:(ci + 1) * 128],
                            a16[ci][:, pg * 256 + hp * 128:pg * 256 + hp * 128 + 128],
                            ident, is_transpose=True,
                            start=(hp == 0 and ci == 0), stop=(hp == 1 and ci == 2),
                            skip_group_check=True)
                t8 = tps.bitcast(F8)  # [128, 2, 768]
                qA = qkTp.tile([128, 2, 2, 128], F8, tag="qA")
                qB = qkTp.tile([128, 2, 2, 64], F8, tag="qB")
                kA = qkTp.tile([128, 2, 2, 128], F8, tag="kA")
                kB = qkTp.tile([128, 2, 2, 64], F8, tag="kB")

                def deint(off, n):
                    return bass.AP(tensor=t8.tensor, offset=t8.offset + off,
                                   ap=[[t8.ap[0][0], 128], [768, 2], [1, 2], [2, n]])
                nc.scalar.copy(out=qA, in_=deint(0, 128))
                nc.vector.tensor_copy(out=qB, in_=deint(256, 64))
                nc.scalar.copy(out=kA, in_=deint(384, 128))
                nc.vector.tensor_copy(out=kB, in_=deint(640, 64))

                for hp in range(2):
                    first_g = (mj == 0 and pg == 0 and hp == 0)
                    last_g = (mj == NMACRO - 1 and pg == PXM // 512 - 1 and hp == 1)
                    nc.tensor.matmul(gram_ps[0:128, 0:128], qA[:, hp], kA[:, hp],
                                     start=first_g, stop=last_g,
                                     perf_mode=DR, skip_group_check=True)
                    nc.tensor.matmul(gram_ps[0:128, 128:192], qA[:, hp], kB[:, hp],
                                     start=False, stop=last_g,
                                     perf_mode=DR, skip_group_check=True)
                    nc.tensor.matmul(gram_ps[0:64, 192:320], qB[:, hp], kA[:, hp],
                                     start=False, stop=last_g,
                                     perf_mode=DR, skip_group_check=True)
                    nc.tensor.matmul(gram_ps[0:64, 320:384], qB[:, hp], kB[:, hp],
                                     start=False, stop=last_g,
                                     perf_mode=DR, skip_group_check=True)
                    nc.tensor.matmul(gram2_ps[0:128, 0:128], qA[:, hp], qA[:, hp],
                                     start=first_g, stop=last_g,
                                     perf_mode=DR, skip_group_check=True)
                    nc.tensor.matmul(gram2_ps[0:64, 128:192], qB[:, hp], qB[:, hp],
                                     start=False, stop=last_g,
                                     perf_mode=DR, skip_group_check=True)
                    nc.tensor.matmul(gram2_ps[0:128, 192:320], kA[:, hp], kA[:, hp],
                                     start=False, stop=last_g,
                                     perf_mode=DR, skip_group_check=True)
                    nc.tensor.matmul(gram2_ps[0:64, 320:384], kB[:, hp], kB[:, hp],
                                     start=False, stop=last_g,
                                     perf_mode=DR, skip_group_check=True)

        gram_sb = consts.tile([128, 384], F32, tag="gsb")
        gram2_sb = consts.tile([128, 384], F32, tag="g2sb")
        nc.vector.memset(gram_sb[64:128, 192:384], 0.0)

        nc.vector.tensor_copy(out=gram_sb[:, 0:192], in_=gram_ps[0:128, 0:192])
        nc.vector.tensor_copy(out=gram_sb[0:64, 192:384],
                              in_=gram_ps[0:64, 192:384])
        nc.vector.tensor_copy(out=gram2_sb, in_=gram2_ps[0:128, 0:384])
        nc.sync.dma_start(out=gram_out, in_=gram_sb)
        nc.sync.dma_start(out=sumsq_out, in_=gram2_sb)
    nc.compile()
    return nc


def _build_phase2():
    nc = bacc.Bacc("TRN2", target_bir_lowering=False, debug=False,
                   num_devices=N_CORES)
    v_in = nc.dram_tensor("v_sp", [C, ROWS * W], F16, kind="ExternalInput").ap()
    mwT = nc.dram_tensor("mwT", [C, C], F16, kind="ExternalInput").ap()
    out_loc = nc.dram_tensor("out_loc", [C, ROWS * W], F16, kind="ExternalOutput").ap()

    BT = 4096
    NT = ROWS * W // BT
    SUB = 512
    with ExitStack() as ctx:
        tc = ctx.enter_context(tile.TileContext(nc))
        consts = ctx.enter_context(tc.tile_pool(name="consts", bufs=1))
        vpool = ctx.enter_context(tc.tile_pool(name="vpool", bufs=4))
        aopool = ctx.enter_context(tc.tile_pool(name="aopool", bufs=4))
        ps_pj = ctx.enter_context(tc.tile_pool(name="ps_pj", bufs=3, space="PSUM"))

        mw = consts.tile([96, 2, C], F16, tag="mw")
        nc.sync.dma_start(out=mw[:, 0, :], in_=mwT[0:96, :])
        nc.sync.dma_start(out=mw[:, 1, :], in_=mwT[96:192, :])

        for t in range(NT):
            px = slice(t * BT, (t + 1) * BT)
            va = vpool.tile([96, BT], F16, tag="va")
            vb = vpool.tile([96, BT], F16, tag="vb")
            nc.sync.dma_start(out=va, in_=v_in[0:96, px])
            nc.sync.dma_start(out=vb, in_=v_in[96:192, px])
            oja = aopool.tile([128, BT], F16, tag="oja")
            ojb = aopool.tile([64, BT], F16, tag="ojb")
            for h in range(BT // SUB):
                hs = slice(h * SUB, (h + 1) * SUB)
                pja = ps_pj.tile([128, SUB], F32, tag="pja")
                pjb = ps_pj.tile([64, SUB], F32, tag="pjb")
                nc.tensor.matmul(pja, mw[:, 0, 0:128], va[:, hs],
                                 start=True, stop=False)
                nc.tensor.matmul(pja, mw[:, 1, 0:128], vb[:, hs],
                                 start=False, stop=True)
                nc.tensor.matmul(pjb, mw[:, 0, 128:192], va[:, hs],
                                 start=True, stop=False)
                nc.tensor.matmul(pjb, mw[:, 1, 128:192], vb[:, hs],
                                 start=False, stop=True)
                nc.scalar.copy(out=oja[:, hs], in_=pja)
                nc.vector.tensor_copy(out=ojb[:, hs], in_=pjb)
            nc.sync.dma_start(out=out_loc[0:128, px], in_=oja)
            nc.sync.dma_start(out=out_loc[128:192, px], in_=ojb)
    nc.compile()
    return nc


_NC1 = None
_NC2 = None
_LAST_R1 = None
_LAST_R2 = None


def _get_programs():
    global _NC1, _NC2
    if _NC1 is None:
        _NC1 = _build_phase1()
        _NC2 = _build_phase2()
    return _NC1, _NC2


def _q8(a):
    return a.astype(NP8)


def kernel(x, qkv_w, dw_w, proj_w, temperature, _trace=False):
    x = np.asarray(x, dtype=np.float32)
    qkv_w = np.asarray(qkv_w, dtype=np.float32)
    dw_w = np.asarray(dw_w, dtype=np.float32)
    proj_w = np.asarray(proj_w, dtype=np.float32)
    temperature = np.asarray(temperature, dtype=np.float32)

    nc1, nc2 = _get_programs()

    # ---- weight prep ----
    qkv_wT = np.ascontiguousarray(qkv_w[:, :, 0, 0].T)     # [192 in, 576 out]
    wT_pad = np.zeros((256, C3), np.float32)
    wT_pad[0:192] = qkv_wT
    w8 = _q8(wT_pad).astype(np.float32)
    w_lo = _q8(wT_pad - w8)
    w_hi = _q8(wT_pad)
    wqk_np = np.zeros((128, 3, 2, 128), NP8)
    for ci in range(3):
        for kt in range(2):
            wqk_np[:, ci, kt] = w_hi[kt * 128:(kt + 1) * 128,
                                     ci * 128:(ci + 1) * 128]
    wv_np = np.zeros((128, 2, 2, 2, 128), NP8)
    for cj, (c0, cw) in enumerate(((384, 128), (512, 64))):
        for kt in range(2):
            wv_np[:, cj, 0, kt, 0:cw] = w_hi[kt * 128:(kt + 1) * 128, c0:c0 + cw]
            wv_np[:, cj, 1, kt, 0:cw] = w_lo[kt * 128:(kt + 1) * 128, c0:c0 + cw]

    dw_flat = np.ascontiguousarray(dw_w[:, 0].reshape(C3, 9))
    d8 = _q8(dw_flat).astype(np.float32)
    d_hi = _q8(dw_flat)
    d_lo = _q8(dw_flat - d8)
    dq_np = np.zeros((128, 3, 5, 2, 128), NP8)
    for ci in range(3):
        for pi, (tA, tB) in enumerate(TAP_PAIRS):
            dA = np.diag(d_hi[ci * 128:(ci + 1) * 128, tA].astype(np.float32))
            dq_np[:, ci, pi, 0] = dA.astype(NP8)
            if tB != tA:
                dBv = np.diag(d_hi[ci * 128:(ci + 1) * 128, tB].astype(np.float32))
                dq_np[:, ci, pi, 1] = dBv.astype(NP8)
    dv_np = np.zeros((128, 2, 5, 2, 128), NP8)
    for ws, dsrc in ((0, d_hi), (1, d_lo)):
        for pi, (tA, tB) in enumerate(TAP_PAIRS):
            dA = np.diag(dsrc[384:512, tA].astype(np.float32))
            dv_np[:, ws, pi, 0] = dA.astype(NP8)
            if tB != tA:
                dBv = np.diag(dsrc[384:512, tB].astype(np.float32))
                dv_np[:, ws, pi, 1] = dBv.astype(NP8)
    dv4_np = np.zeros((64, 2, 5, 2, 64), NP8)
    for ws, dsrc in ((0, d_hi), (1, d_lo)):
        for pi, (tA, tB) in enumerate(TAP_PAIRS):
            dA = np.diag(dsrc[512:576, tA].astype(np.float32))
            dv4_np[:, ws, pi, 0] = dA.astype(NP8)
            if tB != tA:
                dBv = np.diag(dsrc[512:576, tB].astype(np.float32))
                dv4_np[:, ws, pi, 1] = dBv.astype(NP8)
    ident_np = np.eye(128, dtype=np.float16)

    # ---- x prep: hi/lo fp8, kt-padded, halo rows ----
    in_maps1 = []
    for core in range(N_CORES):
        b, half = divmod(core, 2)
        base = half * ROWS
        x_pad = np.zeros((256, ROWS + 2, W), np.float32)
        lo, hi = base - 1, base + ROWS + 1
        slo, shi = max(lo, 0), min(hi, H)
        x_pad[0:192, slo - lo:shi - lo, :] = x[b, :, slo:shi, :]
        xh = _q8(x_pad)
        xl = _q8(x_pad - xh.astype(np.float32))
        x_hl = np.stack([xh, xl], axis=0)
        in_maps1.append({"x_hl": x_hl, "wqk": wqk_np, "wv": wv_np,
                         "dq": dq_np, "dv": dv_np, "dv4": dv4_np,
                         "ident_in": ident_np})

    global _LAST_R1, _LAST_R2
    r1 = run_bass_kernel_spmd(nc1, in_maps1, core_ids=list(range(N_CORES)),
                              trace=_trace)
    _LAST_R1 = r1

    # ---- host: combine partials, softmax, fold proj into attn ----
    proj_w2 = proj_w[:, :, 0, 0].astype(np.float64)
    mwTs = np.zeros((B, C, C), np.float16)
    for b in range(B):
        ra, rb = r1.results[2 * b], r1.results[2 * b + 1]
        gp = ra["gram_part"].astype(np.float64) + rb["gram_part"].astype(np.float64)
        G = np.zeros((192, 192))
        G[0:128, 0:128] = gp[0:128, 0:128]
        G[0:128, 128:192] = gp[0:128, 128:192]
        G[128:192, 0:128] = gp[0:64, 192:320]
        G[128:192, 128:192] = gp[0:64, 320:384]
        g2 = ra["sumsq_part"].astype(np.float64) + rb["sumsq_part"].astype(np.float64)
        qsq = np.concatenate([np.diag(g2[0:128, 0:128]),
                              np.diag(g2[0:64, 128:192])])
        ksq = np.concatenate([np.diag(g2[0:128, 192:320]),
                              np.diag(g2[0:64, 320:384])])
        qn = np.maximum(np.sqrt(qsq), EPS)
        kn = np.maximum(np.sqrt(ksq), EPS)
        attn_bd = np.zeros((C, C))
        for h in range(HEADS):
            sl = slice(h * DH, (h + 1) * DH)
            Gh = G[sl, sl] / np.outer(qn[sl], kn[sl]) * float(temperature[h, 0, 0])
            Gh = Gh - Gh.max(axis=1, keepdims=True)
            e = np.exp(Gh)
            attn_bd[sl, sl] = e / e.sum(axis=1, keepdims=True)
        mwTs[b] = (proj_w2 @ attn_bd).T.astype(np.float16)

    in_maps2 = []
    for core in range(N_CORES):
        b = core // 2
        in_maps2.append({"v_sp": r1.results[core]["v_sp"], "mwT": mwTs[b]})
    r2 = run_bass_kernel_spmd(nc2, in_maps2, core_ids=list(range(N_CORES)),
                              trace=_trace)
    _LAST_R2 = r2

    out = np.zeros((B, C, H, W), np.float32)
    for core in range(N_CORES):
        b, half = divmod(core, 2)
        out[b, :, half * ROWS:(half + 1) * ROWS, :] = \
            r2.results[core]["out_loc"].reshape(C, ROWS, W).astype(np.float32)
    return out


# revision 3
# speedup vs baseline: 1.0338x; 1.0338x over previous
"""Trainium2 Bass kernel for nn_Attention_4415226380830 (XCA channel attention),
restructured around fp8 DoubleRow matmuls.

Dtype strategy (validated vs reference in numpy):
  - q,k path: fully fp8e4m3 (errors wash out in the 65536-px Gram).
  - v path: x and conv weights split hi+lo fp8 (3-term products ~= fp16
    accuracy at 2x fp16 matmul rate); dw for v-chunk3 likewise hi/lo DR
    pairs on PE; v-chunk4 (64 ch) taps on DVE/Pool at fp16.
  - v spill + out: fp16.

PE work per macro (16 rows): conv qk 3xDR + v 2x3xDR, dw as DR *tap pairs*
(2 shifted windows per matmul via a stride-delta kt dim), packed fp16-view
transposes (2 px/row), DR gram (256 px per pass). Norms via ACT Square+accum
on fp8 acc. Evacs split ACT/DVE (Pool cannot read PSUM).
"""

import numpy as np
from contextlib import ExitStack

import ml_dtypes
import concourse.bass as bass
from concourse import bacc
import concourse.mybir as mybir
import concourse.tile as tile
from concourse.bass_utils import run_bass_kernel_spmd

F32 = mybir.dt.float32
F16 = mybir.dt.float16
F8 = mybir.dt.float8e4
NP8 = ml_dtypes.float8_e4m3fn
DR = mybir.MatmulPerfMode.DoubleRow

B = 4
C = 192
HEADS = 4
DH = C // HEADS
H = 256
W = 256
C3 = 3 * C
N_CORES = 8
EPS = 1e-12

ROWS = H // 2            # rows per core
MROWS = 16               # rows per macro-tile
NMACRO = ROWS // MROWS   # 8
WIN_ROWS = MROWS + 2     # 18
PXM = MROWS * W          # 4096

# conv psum substeps over the 18-row window: 4+4+4+4+2 rows
CONV_STEPS = [(r, min(4, WIN_ROWS - r)) for r in range(0, WIN_ROWS, 4)]
# dw substeps: 4 rows = 1024 px each
DWSTEPS = 4
DWROWS = MROWS // DWSTEPS  # 4 rows per substep
# tap pairs (t = 3*(dy+1) + (dx+1)); last pair duplicates tap 8 with zero B
TAP_PAIRS = [(0, 1), (2, 3), (4, 5), (6, 7), (8, 8)]


def _tap_delta(tA, tB):
    dyA, dxA = tA // 3 - 1, tA % 3 - 1
    dyB, dxB = tB // 3 - 1, tB % 3 - 1
    return (dyB - dyA) * 258 + (dxB - dxA)


def _dw_rhs(win, cp, si2, tA, tB):
    """DR moving AP: pair of shifted 2-row windows from win [cp,18,258]."""
    dyA, dxA = tA // 3 - 1, tA % 3 - 1
    base = win[0:cp, 1 + dyA + 2 * si2:1 + dyA + 2 * si2 + 2,
               1 + dxA:257 + dxA]
    return bass.AP(tensor=win.tensor, offset=base.offset,
                   ap=[[win.ap[0][0], cp], [_tap_delta(tA, tB), 2],
                       [258, 2], [1, 256]])


def _build_phase1():
    nc = bacc.Bacc("TRN2", target_bir_lowering=False, debug=False,
                   num_devices=N_CORES)
    # x hi/lo, kt-padded to 256 channels (192:256 zeroed on host)
    x_hl = nc.dram_tensor("x_hl", [2, 256, ROWS + 2, W], F8,
                          kind="ExternalInput").ap()
    wqk_in = nc.dram_tensor("wqk", [128, 3, 2, 128], F8, kind="ExternalInput").ap()
    wv_in = nc.dram_tensor("wv", [128, 2, 2, 2, 128], F8, kind="ExternalInput").ap()
    dq_in = nc.dram_tensor("dq", [128, 3, 5, 2, 128], F8, kind="ExternalInput").ap()
    dv_in = nc.dram_tensor("dv", [128, 2, 5, 2, 128], F8, kind="ExternalInput").ap()
    dv4_in = nc.dram_tensor("dv4", [128, 2, 5, 2, 64], F8, kind="ExternalInput").ap()
    ident_in = nc.dram_tensor("ident_in", [128, 128], F16, kind="ExternalInput").ap()

    gram_out = nc.dram_tensor("gram_part", [128, 384], F32, kind="ExternalOutput").ap()
    sumsq_out = nc.dram_tensor("sumsq_part", [128, 384], F32,
                               kind="ExternalOutput").ap()
    v_out = nc.dram_tensor("v_sp", [C, ROWS * W], F16, kind="ExternalOutput").ap()

    with ExitStack() as ctx:
        tc = ctx.enter_context(tile.TileContext(nc))
        consts = ctx.enter_context(tc.tile_pool(name="consts", bufs=1))
        xpool = ctx.enter_context(tc.tile_pool(name="xpool", bufs=2))
        winp = ctx.enter_context(tc.tile_pool(name="winp", bufs=4))
        winvp = ctx.enter_context(tc.tile_pool(name="winvp", bufs=2))
        accp = ctx.enter_context(tc.tile_pool(name="accp", bufs=4))
        vaccp = ctx.enter_context(tc.tile_pool(name="vaccp", bufs=2))
        qkTp = ctx.enter_context(tc.tile_pool(name="qkTp", bufs=4))
        ps_big = ctx.enter_context(tc.tile_pool(name="ps_big", bufs=4, space="PSUM"))
        ps_t = ctx.enter_context(tc.tile_pool(name="ps_t", bufs=2, space="PSUM"))
        ps_g = ctx.enter_context(tc.tile_pool(name="ps_g", bufs=1, space="PSUM"))
        ps_g2 = ctx.enter_context(tc.tile_pool(name="ps_g2", bufs=1, space="PSUM"))

        wqk = consts.tile([128, 3, 2, 128], F8, tag="wqk")
        wv = consts.tile([128, 2, 2, 2, 128], F8, tag="wv")
        dq = consts.tile([128, 3, 5, 2, 128], F8, tag="dq")
        dv = consts.tile([128, 2, 5, 2, 128], F8, tag="dv")
        dv4 = consts.tile([128, 2, 5, 2, 64], F8, tag="dv4")
        ident = consts.tile([128, 128], F16, tag="ident")
        nc.sync.dma_start(out=wqk, in_=wqk_in)
        nc.sync.dma_start(out=wv, in_=wv_in)
        nc.sync.dma_start(out=dq, in_=dq_in)
        nc.sync.dma_start(out=dv, in_=dv_in)
        nc.sync.dma_start(out=dv4, in_=dv4_in)
        nc.sync.dma_start(out=ident, in_=ident_in)

        gram_ps = ps_g.tile([128, 384], F32)
        gram2_ps = ps_g2.tile([128, 384], F32)

        for mj in range(NMACRO):
            r0 = MROWS * mj
            # x tile [128, kt, hl, 18, 256]
            xt = xpool.tile([128, 2, 2, WIN_ROWS, W], F8, tag="xt")
            for kt in range(2):
                for hl in range(2):
                    nc.sync.dma_start(
                        out=xt[:, kt, hl],
                        in_=x_hl[hl, 128 * kt:128 * (kt + 1), r0:r0 + WIN_ROWS, :])

            accs = {}
            wins = {}

            def conv_qk(ci):
                win = winp.tile([128, WIN_ROWS, 258], F8, tag="win")
                nc.gpsimd.memset(win[:, :, 0:258:257], 0.0)
                for st in range(WIN_ROWS // 2):
                    sr = 2 * st
                    pc = ps_big.tile([128, 512], F32, tag="pw")
                    nc.tensor.matmul(pc, wqk[:, ci],
                                     xt[:, :, 0, sr:sr + 2, :],
                                     start=True, stop=True, perf_mode=DR)
                    dst = win[:, sr:sr + 2, 1:257]
                    srcc = pc.rearrange("p (r w) -> p r w", w=W)
                    if st % 2 == 0:
                        nc.scalar.copy(out=dst, in_=srcc)
                    else:
                        nc.vector.tensor_copy(out=dst, in_=srcc)
                wins[ci] = win

            def conv_v(cj):
                cp = 128 if cj == 0 else 64
                if cj == 0:
                    wh = winvp.tile([128, WIN_ROWS, 258], F8, tag="wh0")
                    wl = winvp.tile([128, WIN_ROWS, 258], F8, tag="wl0")
                else:
                    whl = winvp.tile([128, WIN_ROWS, 258], F8, tag="whl1")
                    wh, wl = whl[0:64], whl[64:128]
                nc.gpsimd.memset(wh[:, :, 0:258:257], 0.0)
                nc.gpsimd.memset(wl[:, :, 0:258:257], 0.0)
                for st in range(WIN_ROWS // 2):
                    sr = 2 * st
                    rs = slice(sr, sr + 2)
                    pc = ps_big.tile([128, 512], F32, tag="pw")
                    nc.tensor.matmul(pc[0:cp], wv[:, cj, 0, :, 0:cp],
                                     xt[:, :, 0, rs, :],
                                     start=True, stop=False, perf_mode=DR)
                    nc.tensor.matmul(pc[0:cp], wv[:, cj, 0, :, 0:cp],
                                     xt[:, :, 1, rs, :],
                                     start=False, stop=False, perf_mode=DR)
                    nc.tensor.matmul(pc[0:cp], wv[:, cj, 1, :, 0:cp],
                                     xt[:, :, 0, rs, :],
                                     start=False, stop=True, perf_mode=DR)
                    srcv = pc[0:cp].rearrange("p (r w) -> p r w", w=W)
                    nc.scalar.copy(out=wh[:, rs, 1:257], in_=srcv)
                    nc.vector.tensor_tensor(out=wl[:, rs, 1:257],
                                            in0=srcv, in1=wh[:, rs, 1:257],
                                            op=mybir.AluOpType.subtract)
                wins[3 + cj] = (wh, wl, cp) if cj == 0 else (whl, wl, cp)

            def dw_qk(ci):
                acc = accp.tile([128, PXM], F8, tag="acc")
                for si2 in range(2 * DWSTEPS):
                    pd = ps_big.tile([128, 512], F32, tag="pw")
                    for pi, (tA, tB) in enumerate(TAP_PAIRS):
                        nc.tensor.matmul(pd, dq[:, ci, pi],
                                         _dw_rhs(wins[ci], 128, si2, tA, tB),
                                         start=(pi == 0), stop=(pi == 4),
                                         perf_mode=DR)
                    if si2 % 2 == 0:
                        nc.scalar.copy(out=acc[:, si2 * 512:(si2 + 1) * 512], in_=pd)
                    else:
                        nc.vector.tensor_copy(out=acc[:, si2 * 512:(si2 + 1) * 512],
                                              in_=pd)
                accs[ci] = acc

            def dw_v(cj, vacc):
                wh, wl, cp = wins[3 + cj]
                if cj == 0:
                    plan = [(dv, 0, wh), (dv, 0, wl), (dv, 1, wh)]
                    nlast = 14
                else:
                    # wh is the packed [128] hi/lo tile; dv4 set0 handles h+l
                    plan = [(dv4, 0, wh), (dv4, 1, wh)]
                    nlast = 9
                for si2 in range(2 * DWSTEPS):
                    pd = ps_big.tile([128, 512], F32, tag="pw")
                    k = 0
                    for (dvt, wset, wn) in plan:
                        for pi, (tA, tB) in enumerate(TAP_PAIRS):
                            nc.tensor.matmul(pd[0:cp],
                                             dvt[:, wset, pi],
                                             _dw_rhs(wn, 128, si2, tA, tB),
                                             start=(k == 0), stop=(k == nlast),
                                             perf_mode=DR)
                            k += 1
                    if si2 % 2 == 0:
                        nc.scalar.copy(out=vacc[:, si2 * 512:(si2 + 1) * 512],
                                       in_=pd[0:cp])
                    else:
                        nc.vector.tensor_copy(out=vacc[:, si2 * 512:(si2 + 1) * 512],
                                              in_=pd[0:cp])

            vacc3 = vaccp.tile([128, PXM], F16, tag="v3")
            vacc4 = vaccp.tile([64, PXM], F16, tag="v4")
            # software pipeline: conv(i+1) issues before dw(i)
            conv_qk(0)
            conv_qk(1)
            dw_qk(0)
            conv_qk(2)
            dw_qk(1)
            conv_v(0)
            dw_qk(2)
            conv_v(1)
            dw_v(0, vacc3)
            dw_v(1, vacc4)

            # ======== Phase C: spill v, transposes, grams ========
            nc.sync.dma_start(out=v_out[0:128, mj * PXM:(mj + 1) * PXM], in_=vacc3)
            nc.sync.dma_start(out=v_out[128:192, mj * PXM:(mj + 1) * PXM], in_=vacc4)

            a16 = [accs[ci].bitcast(F16) for ci in range(3)]   # [128, 2048]
            for pg in range(PXM // 512):    # 2 pr per group
                tps = ps_t.tile([128, 2, 384], F16, tag="tps")
                for hp in range(2):
                    for ci in range(3):
                        nc.tensor.matmul(
                            tps[:, hp, ci * 128:(ci + 1) * 128],
                            a16[ci][:, pg * 256 + hp * 128:pg * 256 + hp * 128 + 128],
                            ident, is_transpose=True,
                            start=(hp == 0 and ci == 0), stop=(hp == 1 and ci == 2),
                            skip_group_check=True)
                t8 = tps.bitcast(F8)  # [128, 2, 768]
                qA = qkTp.tile([128, 2, 2, 128], F8, tag="qA")
                qB = qkTp.tile([128, 2, 2, 64], F8, tag="qB")
                kA = qkTp.tile([128, 2, 2, 128], F8, tag="kA")
                kB = qkTp.tile([128, 2, 2, 64], F8, tag="kB")

                def deint(off, n):
                    return bass.AP(tensor=t8.tensor, offset=t8.offset + off,
                                   ap=[[t8.ap[0][0], 128], [768, 2], [1, 2], [2, n]])
                nc.scalar.copy(out=qA, in_=deint(0, 128))
                nc.vector.tensor_copy(out=qB, in_=deint(256, 64))
                nc.scalar.copy(out=kA, in_=deint(384, 128))
                nc.vector.tensor_copy(out=kB, in_=deint(640, 64))

                for hp in range(2):
                    first_g = (mj == 0 and pg == 0 and hp == 0)
                    last_g = (mj == NMACRO - 1 and pg == PXM // 512 - 1 and hp == 1)
                    nc.tensor.matmul(gram_ps[0:128, 0:128], qA[:, hp], kA[:, hp],
                                     start=first_g, stop=last_g,
                                     perf_mode=DR, skip_group_check=True)
                    nc.tensor.matmul(gram_ps[0:128, 128:192], qA[:, hp], kB[:, hp],
                                     start=False, stop=last_g,
                                     perf_mode=DR, skip_group_check=True)
                    nc.tensor.matmul(gram_ps[0:64, 192:320], qB[:, hp], kA[:, hp],
                                     start=False, stop=last_g,
                                     perf_mode=DR, skip_group_check=True)
                    nc.tensor.matmul(gram_ps[0:64, 320:384], qB[:, hp], kB[:, hp],
                                     start=False, stop=last_g,
                                     perf_mode=DR, skip_group_check=True)
                    nc.tensor.matmul(gram2_ps[0:128, 0:128], qA[:, hp], qA[:, hp],
                                     start=first_g, stop=last_g,
                                     perf_mode=DR, skip_group_check=True)
                    nc.tensor.matmul(gram2_ps[0:64, 128:192], qB[:, hp], qB[:, hp],
                                     start=False, stop=last_g,
                                     perf_mode=DR, skip_group_check=True)
                    nc.tensor.matmul(gram2_ps[0:128, 192:320], kA[:, hp], kA[:, hp],
                                     start=False, stop=last_g,
                                     perf_mode=DR, skip_group_check=True)
                    nc.tensor.matmul(gram2_ps[0:64, 320:384], kB[:, hp], kB[:, hp],
                                     start=False, stop=last_g,
                                     perf_mode=DR, skip_group_check=True)

        gram_sb = consts.tile([128, 384], F32, tag="gsb")
        gram2_sb = consts.tile([128, 384], F32, tag="g2sb")
        nc.vector.memset(gram_sb[64:128, 192:384], 0.0)

        nc.vector.tensor_copy(out=gram_sb[:, 0:192], in_=gram_ps[0:128, 0:192])
        nc.vector.tensor_copy(out=gram_sb[0:64, 192:384],
                              in_=gram_ps[0:64, 192:384])
        nc.vector.tensor_copy(out=gram2_sb, in_=gram2_ps[0:128, 0:384])
        nc.sync.dma_start(out=gram_out, in_=gram_sb)
        nc.sync.dma_start(out=sumsq_out, in_=gram2_sb)
    nc.compile()
    return nc


def _build_phase2():
    nc = bacc.Bacc("TRN2", target_bir_lowering=False, debug=False,
                   num_devices=N_CORES)
    v_in = nc.dram_tensor("v_sp", [C, ROWS * W], F16, kind="ExternalInput").ap()
    mwT = nc.dram_tensor("mwT", [C, C], F16, kind="ExternalInput").ap()
    out_loc = nc.dram_tensor("out_loc", [C, ROWS * W], F16, kind="ExternalOutput").ap()

    BT = 4096
    NT = ROWS * W // BT
    SUB = 512
    with ExitStack() as ctx:
        tc = ctx.enter_context(tile.TileContext(nc))
        consts = ctx.enter_context(tc.tile_pool(name="consts", bufs=1))
        vpool = ctx.enter_context(tc.tile_pool(name="vpool", bufs=6))
        aopool = ctx.enter_context(tc.tile_pool(name="aopool", bufs=6))
        ps_pj = ctx.enter_context(tc.tile_pool(name="ps_pj", bufs=3, space="PSUM"))

        mw = consts.tile([96, 2, C], F16, tag="mw")
        nc.sync.dma_start(out=mw[:, 0, :], in_=mwT[0:96, :])
        nc.sync.dma_start(out=mw[:, 1, :], in_=mwT[96:192, :])

        for t in range(NT):
            px = slice(t * BT, (t + 1) * BT)
            va = vpool.tile([96, BT], F16, tag="va")
            vb = vpool.tile([96, BT], F16, tag="vb")
            nc.sync.dma_start(out=va, in_=v_in[0:96, px])
            nc.sync.dma_start(out=vb, in_=v_in[96:192, px])
            oja = aopool.tile([128, BT], F16, tag="oja")
            ojb = aopool.tile([64, BT], F16, tag="ojb")
            for h in range(BT // SUB):
                hs = slice(h * SUB, (h + 1) * SUB)
                pja = ps_pj.tile([128, SUB], F32, tag="pja")
                pjb = ps_pj.tile([64, SUB], F32, tag="pjb")
                nc.tensor.matmul(pja, mw[:, 0, 0:128], va[:, hs],
                                 start=True, stop=False)
                nc.tensor.matmul(pja, mw[:, 1, 0:128], vb[:, hs],
                                 start=False, stop=True)
                nc.tensor.matmul(pjb, mw[:, 0, 128:192], va[:, hs],
                                 start=True, stop=False)
                nc.tensor.matmul(pjb, mw[:, 1, 128:192], vb[:, hs],
                                 start=False, stop=True)
                nc.scalar.copy(out=oja[:, hs], in_=pja)
                nc.vector.tensor_copy(out=ojb[:, hs], in_=pjb)
            nc.sync.dma_start(out=out_loc[0:128, px], in_=oja)
            nc.sync.dma_start(out=out_loc[128:192, px], in_=ojb)
    nc.compile()
    return nc


_NC1 = None
_NC2 = None
_LAST_R1 = None
_LAST_R2 = None


def _get_programs():
    global _NC1, _NC2
    if _NC1 is None:
        _NC1 = _build_phase1()
        _NC2 = _build_phase2()
    return _NC1, _NC2


def _q8(a):
    return a.astype(NP8)


def kernel(x, qkv_w, dw_w, proj_w, temperature, _trace=False):
    x = np.asarray(x, dtype=np.float32)
    qkv_w = np.asarray(qkv_w, dtype=np.float32)
    dw_w = np.asarray(dw_w, dtype=np.float32)
    proj_w = np.asarray(proj_w, dtype=np.float32)
    temperature = np.asarray(temperature, dtype=np.float32)

    nc1, nc2 = _get_programs()

    # ---- weight prep ----
    qkv_wT = np.ascontiguousarray(qkv_w[:, :, 0, 0].T)     # [192 in, 576 out]
    wT_pad = np.zeros((256, C3), np.float32)
    wT_pad[0:192] = qkv_wT
    w8 = _q8(wT_pad).astype(np.float32)
    w_lo = _q8(wT_pad - w8)
    w_hi = _q8(wT_pad)
    wqk_np = np.zeros((128, 3, 2, 128), NP8)
    for ci in range(3):
        for kt in range(2):
            wqk_np[:, ci, kt] = w_hi[kt * 128:(kt + 1) * 128,
                                     ci * 128:(ci + 1) * 128]
    wv_np = np.zeros((128, 2, 2, 2, 128), NP8)
    for cj, (c0, cw) in enumerate(((384, 128), (512, 64))):
        for kt in range(2):
            wv_np[:, cj, 0, kt, 0:cw] = w_hi[kt * 128:(kt + 1) * 128, c0:c0 + cw]
            wv_np[:, cj, 1, kt, 0:cw] = w_lo[kt * 128:(kt + 1) * 128, c0:c0 + cw]

    dw_flat = np.ascontiguousarray(dw_w[:, 0].reshape(C3, 9))
    d8 = _q8(dw_flat).astype(np.float32)
    d_hi = _q8(dw_flat)
    d_lo = _q8(dw_flat - d8)
    dq_np = np.zeros((128, 3, 5, 2, 128), NP8)
    for ci in range(3):
        for pi, (tA, tB) in enumerate(TAP_PAIRS):
            dA = np.diag(d_hi[ci * 128:(ci + 1) * 128, tA].astype(np.float32))
            dq_np[:, ci, pi, 0] = dA.astype(NP8)
            if tB != tA:
                dBv = np.diag(d_hi[ci * 128:(ci + 1) * 128, tB].astype(np.float32))
                dq_np[:, ci, pi, 1] = dBv.astype(NP8)
    dv_np = np.zeros((128, 2, 5, 2, 128), NP8)
    for ws, dsrc in ((0, d_hi), (1, d_lo)):
        for pi, (tA, tB) in enumerate(TAP_PAIRS):
            dA = np.diag(dsrc[384:512, tA].astype(np.float32))
            dv_np[:, ws, pi, 0] = dA.astype(NP8)
            if tB != tA:
                dBv = np.diag(dsrc[384:512, tB].astype(np.float32))
                dv_np[:, ws, pi, 1] = dBv.astype(NP8)
    dv4_np = np.zeros((128, 2, 5, 2, 64), NP8)
    dh4 = d_hi[512:576].astype(np.float32)
    dl4 = d_lo[512:576].astype(np.float32)
    for pi, (tA, tB) in enumerate(TAP_PAIRS):
        for kt, t in ((0, tA), (1, tB)):
            if kt == 1 and tB == tA:
                continue
            dv4_np[0:64, 0, pi, kt] = np.diag(dh4[:, t]).astype(NP8)
            dv4_np[64:128, 0, pi, kt] = np.diag(dh4[:, t]).astype(NP8)
            dv4_np[0:64, 1, pi, kt] = np.diag(dl4[:, t]).astype(NP8)
    ident_np = np.eye(128, dtype=np.float16)

    # ---- x prep: hi/lo fp8, kt-padded, halo rows ----
    in_maps1 = []
    for core in range(N_CORES):
        b, half = divmod(core, 2)
        base = half * ROWS
        x_pad = np.zeros((256, ROWS + 2, W), np.float32)
        lo, hi = base - 1, base + ROWS + 1
        slo, shi = max(lo, 0), min(hi, H)
        x_pad[0:192, slo - lo:shi - lo, :] = x[b, :, slo:shi, :]
        xh = _q8(x_pad)
        xl = _q8(x_pad - xh.astype(np.float32))
        x_hl = np.stack([xh, xl], axis=0)
        in_maps1.append({"x_hl": x_hl, "wqk": wqk_np, "wv": wv_np,
                         "dq": dq_np, "dv": dv_np, "dv4": dv4_np,
                         "ident_in": ident_np})

    global _LAST_R1, _LAST_R2
    r1 = run_bass_kernel_spmd(nc1, in_maps1, core_ids=list(range(N_CORES)),
                              trace=_trace)
    _LAST_R1 = r1

    # ---- host: combine partials, softmax, fold proj into attn ----
    proj_w2 = proj_w[:, :, 0, 0].astype(np.float64)
    mwTs = np.zeros((B, C, C), np.float16)
    for b in range(B):
        ra, rb = r1.results[2 * b], r1.results[2 * b + 1]
        gp = ra["gram_part"].astype(np.float64) + rb["gram_part"].astype(np.float64)
        G = np.zeros((192, 192))
        G[0:128, 0:128] = gp[0:128, 0:128]
        G[0:128, 128:192] = gp[0:128, 128:192]
        G[128:192, 0:128] = gp[0:64, 192:320]
        G[128:192, 128:192] = gp[0:64, 320:384]
        g2 = ra["sumsq_part"].astype(np.float64) + rb["sumsq_part"].astype(np.float64)
        qsq = np.concatenate([np.diag(g2[0:128, 0:128]),
                              np.diag(g2[0:64, 128:192])])
        ksq = np.concatenate([np.diag(g2[0:128, 192:320]),
                              np.diag(g2[0:64, 320:384])])
        qn = np.maximum(np.sqrt(qsq), EPS)
        kn = np.maximum(np.sqrt(ksq), EPS)
        attn_bd = np.zeros((C, C))
        for h in range(HEADS):
            sl = slice(h * DH, (h + 1) * DH)
            Gh = G[sl, sl] / np.outer(qn[sl], kn[sl]) * float(temperature[h, 0, 0])
            Gh = Gh - Gh.max(axis=1, keepdims=True)
            e = np.exp(Gh)
            attn_bd[sl, sl] = e / e.sum(axis=1, keepdims=True)
        mwTs[b] = (proj_w2 @ attn_bd).T.astype(np.float16)

    in_maps2 = []
    for core in range(N_CORES):
        b = core // 2
        in_maps2.append({"v_sp": r1.results[core]["v_sp"], "mwT": mwTs[b]})
    r2 = run_bass_kernel_spmd(nc2, in_maps2, core_ids=list(range(N_CORES)),
                              trace=_trace)
    _LAST_R2 = r2

    out = np.zeros((B, C, H, W), np.float32)
    for core in range(N_CORES):
        b, half = divmod(core, 2)
        out[b, :, half * ROWS:(half + 1) * ROWS, :] = \
            r2.results[core]["out_loc"].reshape(C, ROWS, W).astype(np.float32)
    return out


# revision 4
# speedup vs baseline: 1.0562x; 1.0216x over previous
"""Trainium2 Bass kernel for nn_Attention_4415226380830 (XCA channel attention),
restructured around fp8 DoubleRow matmuls.

Dtype strategy (validated vs reference in numpy):
  - q,k path: fully fp8e4m3 (errors wash out in the 65536-px Gram).
  - v path: x and conv weights split hi+lo fp8 (3-term products ~= fp16
    accuracy at 2x fp16 matmul rate); dw for v-chunk3 likewise hi/lo DR
    pairs on PE; v-chunk4 (64 ch) taps on DVE/Pool at fp16.
  - v spill + out: fp16.

PE work per macro (16 rows): conv qk 3xDR + v 2x3xDR, dw as DR *tap pairs*
(2 shifted windows per matmul via a stride-delta kt dim), packed fp16-view
transposes (2 px/row), DR gram (256 px per pass). Norms via ACT Square+accum
on fp8 acc. Evacs split ACT/DVE (Pool cannot read PSUM).
"""

import numpy as np
from contextlib import ExitStack

import ml_dtypes
import concourse.bass as bass
from concourse import bacc
import concourse.mybir as mybir
import concourse.tile as tile
from concourse.bass_utils import run_bass_kernel_spmd

F32 = mybir.dt.float32
F16 = mybir.dt.float16
F8 = mybir.dt.float8e4
NP8 = ml_dtypes.float8_e4m3fn
DR = mybir.MatmulPerfMode.DoubleRow

B = 4
C = 192
HEADS = 4
DH = C // HEADS
H = 256
W = 256
C3 = 3 * C
N_CORES = 8
EPS = 1e-12

ROWS = H // 2            # rows per core
MROWS = 16               # rows per macro-tile
NMACRO = ROWS // MROWS   # 8
WIN_ROWS = MROWS + 2     # 18
PXM = MROWS * W          # 4096

# conv psum substeps over the 18-row window: 4+4+4+4+2 rows
CONV_STEPS = [(r, min(4, WIN_ROWS - r)) for r in range(0, WIN_ROWS, 4)]
# dw substeps: 4 rows = 1024 px each
DWSTEPS = 4
DWROWS = MROWS // DWSTEPS  # 4 rows per substep
# tap pairs (t = 3*(dy+1) + (dx+1)); last pair duplicates tap 8 with zero B
TAP_PAIRS = [(0, 1), (2, 3), (4, 5), (6, 7), (8, 8)]


def _tap_delta(tA, tB):
    dyA, dxA = tA // 3 - 1, tA % 3 - 1
    dyB, dxB = tB // 3 - 1, tB % 3 - 1
    return (dyB - dyA) * 258 + (dxB - dxA)


def _dw_rhs(win, cp, si2, tA, tB):
    """DR moving AP: pair of shifted 2-row windows from win [cp,18,258]."""
    dyA, dxA = tA // 3 - 1, tA % 3 - 1
    base = win[0:cp, 1 + dyA + 2 * si2:1 + dyA + 2 * si2 + 2,
               1 + dxA:257 + dxA]
    return bass.AP(tensor=win.tensor, offset=base.offset,
                   ap=[[win.ap[0][0], cp], [_tap_delta(tA, tB), 2],
                       [258, 2], [1, 256]])


def _build_phase1():
    nc = bacc.Bacc("TRN2", target_bir_lowering=False, debug=False,
                   num_devices=N_CORES)
    # x hi/lo, kt-padded to 256 channels (192:256 zeroed on host)
    x_hl = nc.dram_tensor("x_hl", [2, 256, ROWS + 2, W], F8,
                          kind="ExternalInput").ap()
    wqk_in = nc.dram_tensor("wqk", [128, 3, 2, 128], F8, kind="ExternalInput").ap()
    wv_in = nc.dram_tensor("wv", [128, 2, 2, 2, 128], F8, kind="ExternalInput").ap()
    dq_in = nc.dram_tensor("dq", [128, 3, 5, 2, 128], F8, kind="ExternalInput").ap()
    dv_in = nc.dram_tensor("dv", [128, 2, 5, 2, 128], F8, kind="ExternalInput").ap()
    dv4_in = nc.dram_tensor("dv4", [128, 2, 5, 2, 64], F8, kind="ExternalInput").ap()
    ident_in = nc.dram_tensor("ident_in", [128, 128], F16, kind="ExternalInput").ap()

    gram_out = nc.dram_tensor("gram_part", [128, 384], F32, kind="ExternalOutput").ap()
    sumsq_out = nc.dram_tensor("sumsq_part", [128, 384], F32,
                               kind="ExternalOutput").ap()
    v_out = nc.dram_tensor("v_sp", [C, ROWS * W], F16, kind="ExternalOutput").ap()

    with ExitStack() as ctx:
        tc = ctx.enter_context(tile.TileContext(nc))
        consts = ctx.enter_context(tc.tile_pool(name="consts", bufs=1))
        xpool = ctx.enter_context(tc.tile_pool(name="xpool", bufs=3))
        winp = ctx.enter_context(tc.tile_pool(name="winp", bufs=4))
        winvp = ctx.enter_context(tc.tile_pool(name="winvp", bufs=3))
        accp = ctx.enter_context(tc.tile_pool(name="accp", bufs=5))
        vaccp = ctx.enter_context(tc.tile_pool(name="vaccp", bufs=3))
        qkTp = ctx.enter_context(tc.tile_pool(name="qkTp", bufs=4))
        ps_big = ctx.enter_context(tc.tile_pool(name="ps_big", bufs=4, space="PSUM"))
        ps_t = ctx.enter_context(tc.tile_pool(name="ps_t", bufs=2, space="PSUM"))
        ps_g = ctx.enter_context(tc.tile_pool(name="ps_g", bufs=1, space="PSUM"))
        ps_g2 = ctx.enter_context(tc.tile_pool(name="ps_g2", bufs=1, space="PSUM"))

        wqk = consts.tile([128, 3, 2, 128], F8, tag="wqk")
        wv = consts.tile([128, 2, 2, 2, 128], F8, tag="wv")
        dq = consts.tile([128, 3, 5, 2, 128], F8, tag="dq")
        dv = consts.tile([128, 2, 5, 2, 128], F8, tag="dv")
        dv4 = consts.tile([128, 2, 5, 2, 64], F8, tag="dv4")
        ident = consts.tile([128, 128], F16, tag="ident")
        nc.sync.dma_start(out=wqk, in_=wqk_in)
        nc.sync.dma_start(out=wv, in_=wv_in)
        nc.sync.dma_start(out=dq, in_=dq_in)
        nc.sync.dma_start(out=dv, in_=dv_in)
        nc.sync.dma_start(out=dv4, in_=dv4_in)
        nc.sync.dma_start(out=ident, in_=ident_in)

        gram_ps = ps_g.tile([128, 384], F32)
        gram2_ps = ps_g2.tile([128, 384], F32)

        for mj in range(NMACRO):
            r0 = MROWS * mj
            # x tile [128, kt, hl, 18, 256]
            xt = xpool.tile([128, 2, 2, WIN_ROWS, W], F8, tag="xt")
            for kt in range(2):
                for hl in range(2):
                    nc.sync.dma_start(
                        out=xt[:, kt, hl],
                        in_=x_hl[hl, 128 * kt:128 * (kt + 1), r0:r0 + WIN_ROWS, :])

            accs = {}
            wins = {}

            def conv_qk(ci):
                win = winp.tile([128, WIN_ROWS, 258], F8, tag="win")
                nc.gpsimd.memset(win[:, :, 0:258:257], 0.0)
                for st in range(WIN_ROWS // 2):
                    sr = 2 * st
                    pc = ps_big.tile([128, 512], F32, tag="pw")
                    nc.tensor.matmul(pc, wqk[:, ci],
                                     xt[:, :, 0, sr:sr + 2, :],
                                     start=True, stop=True, perf_mode=DR)
                    dst = win[:, sr:sr + 2, 1:257]
                    srcc = pc.rearrange("p (r w) -> p r w", w=W)
                    if st % 2 == 0:
                        nc.scalar.copy(out=dst, in_=srcc)
                    else:
                        nc.vector.tensor_copy(out=dst, in_=srcc)
                wins[ci] = win

            def conv_v(cj):
                cp = 128 if cj == 0 else 64
                if cj == 0:
                    wh = winvp.tile([128, WIN_ROWS, 258], F8, tag="wh0")
                    wl = winvp.tile([128, WIN_ROWS, 258], F8, tag="wl0")
                else:
                    whl = winvp.tile([128, WIN_ROWS, 258], F8, tag="whl1")
                    wh, wl = whl[0:64], whl[64:128]
                nc.gpsimd.memset(wh[:, :, 0:258:257], 0.0)
                nc.gpsimd.memset(wl[:, :, 0:258:257], 0.0)
                for st in range(WIN_ROWS // 2):
                    sr = 2 * st
                    rs = slice(sr, sr + 2)
                    pc = ps_big.tile([128, 512], F32, tag="pw")
                    nc.tensor.matmul(pc[0:cp], wv[:, cj, 0, :, 0:cp],
                                     xt[:, :, 0, rs, :],
                                     start=True, stop=False, perf_mode=DR)
                    nc.tensor.matmul(pc[0:cp], wv[:, cj, 0, :, 0:cp],
                                     xt[:, :, 1, rs, :],
                                     start=False, stop=False, perf_mode=DR)
                    nc.tensor.matmul(pc[0:cp], wv[:, cj, 1, :, 0:cp],
                                     xt[:, :, 0, rs, :],
                                     start=False, stop=True, perf_mode=DR)
                    srcv = pc[0:cp].rearrange("p (r w) -> p r w", w=W)
                    nc.scalar.copy(out=wh[:, rs, 1:257], in_=srcv)
                    nc.vector.tensor_tensor(out=wl[:, rs, 1:257],
                                            in0=srcv, in1=wh[:, rs, 1:257],
                                            op=mybir.AluOpType.subtract)
                wins[3 + cj] = (wh, wl, cp) if cj == 0 else (whl, wl, cp)

            def dw_qk(ci):
                acc = accp.tile([128, PXM], F8, tag="acc")
                for si2 in range(2 * DWSTEPS):
                    pd = ps_big.tile([128, 512], F32, tag="pw")
                    for pi, (tA, tB) in enumerate(TAP_PAIRS):
                        nc.tensor.matmul(pd, dq[:, ci, pi],
                                         _dw_rhs(wins[ci], 128, si2, tA, tB),
                                         start=(pi == 0), stop=(pi == 4),
                                         perf_mode=DR)
                    if si2 % 2 == 0:
                        nc.scalar.copy(out=acc[:, si2 * 512:(si2 + 1) * 512], in_=pd)
                    else:
                        nc.vector.tensor_copy(out=acc[:, si2 * 512:(si2 + 1) * 512],
                                              in_=pd)
                accs[ci] = acc

            def dw_v(cj, vacc):
                wh, wl, cp = wins[3 + cj]
                if cj == 0:
                    plan = [(dv, 0, wh), (dv, 0, wl), (dv, 1, wh)]
                    nlast = 14
                else:
                    # wh is the packed [128] hi/lo tile; dv4 set0 handles h+l
                    plan = [(dv4, 0, wh), (dv4, 1, wh)]
                    nlast = 9
                for si2 in range(2 * DWSTEPS):
                    pd = ps_big.tile([128, 512], F32, tag="pw")
                    k = 0
                    for (dvt, wset, wn) in plan:
                        for pi, (tA, tB) in enumerate(TAP_PAIRS):
                            nc.tensor.matmul(pd[0:cp],
                                             dvt[:, wset, pi],
                                             _dw_rhs(wn, 128, si2, tA, tB),
                                             start=(k == 0), stop=(k == nlast),
                                             perf_mode=DR)
                            k += 1
                    if si2 % 2 == 0:
                        nc.scalar.copy(out=vacc[:, si2 * 512:(si2 + 1) * 512],
                                       in_=pd[0:cp])
                    else:
                        nc.vector.tensor_copy(out=vacc[:, si2 * 512:(si2 + 1) * 512],
                                              in_=pd[0:cp])

            vacc3 = vaccp.tile([128, PXM], F16, tag="v3")
            vacc4 = vaccp.tile([64, PXM], F16, tag="v4")
            # software pipeline: conv(i+1) issues before dw(i)
            conv_qk(0)
            conv_qk(1)
            dw_qk(0)
            conv_qk(2)
            dw_qk(1)
            conv_v(0)
            dw_qk(2)
            conv_v(1)

            def dw_v_tail():
                dw_v(0, vacc3)
                dw_v(1, vacc4)
                nc.sync.dma_start(out=v_out[0:128, mj * PXM:(mj + 1) * PXM],
                                  in_=vacc3)
                nc.sync.dma_start(out=v_out[128:192, mj * PXM:(mj + 1) * PXM],
                                  in_=vacc4)

            # ======== Phase C: transposes/grams overlap dw_v ========
            a16 = [accs[ci].bitcast(F16) for ci in range(3)]   # [128, 2048]
            for pg in range(PXM // 512):    # 2 pr per group
                tps = ps_t.tile([128, 2, 384], F16, tag="tps")
                for hp in range(2):
                    for ci in range(3):
                        nc.tensor.matmul(
                            tps[:, hp, ci * 128:(ci + 1) * 128],
                            a16[ci][:, pg * 256 + hp * 128:pg * 256 + hp * 128 + 128],
                            ident, is_transpose=True,
                            start=(hp == 0 and ci == 0), stop=(hp == 1 and ci == 2),
                            skip_group_check=True)
                t8 = tps.bitcast(F8)  # [128, 2, 768]
                qA = qkTp.tile([128, 2, 2, 128], F8, tag="qA")
                qB = qkTp.tile([128, 2, 2, 64], F8, tag="qB")
                kA = qkTp.tile([128, 2, 2, 128], F8, tag="kA")
                kB = qkTp.tile([128, 2, 2, 64], F8, tag="kB")

                def deint(off, n):
                    return bass.AP(tensor=t8.tensor, offset=t8.offset + off,
                                   ap=[[t8.ap[0][0], 128], [768, 2], [1, 2], [2, n]])
                nc.scalar.copy(out=qA, in_=deint(0, 128))
                nc.vector.tensor_copy(out=qB, in_=deint(256, 64))
                nc.scalar.copy(out=kA, in_=deint(384, 128))
                nc.vector.tensor_copy(out=kB, in_=deint(640, 64))

                for hp in range(2):
                    first_g = (mj == 0 and pg == 0 and hp == 0)
                    last_g = (mj == NMACRO - 1 and pg == PXM // 512 - 1 and hp == 1)
                    nc.tensor.matmul(gram_ps[0:128, 0:128], qA[:, hp], kA[:, hp],
                                     start=first_g, stop=last_g,
                                     perf_mode=DR, skip_group_check=True)
                    nc.tensor.matmul(gram_ps[0:128, 128:192], qA[:, hp], kB[:, hp],
                                     start=False, stop=last_g,
                                     perf_mode=DR, skip_group_check=True)
                    nc.tensor.matmul(gram_ps[0:64, 192:320], qB[:, hp], kA[:, hp],
                                     start=False, stop=last_g,
                                     perf_mode=DR, skip_group_check=True)
                    nc.tensor.matmul(gram_ps[0:64, 320:384], qB[:, hp], kB[:, hp],
                                     start=False, stop=last_g,
                                     perf_mode=DR, skip_group_check=True)
                    nc.tensor.matmul(gram2_ps[0:128, 0:128], qA[:, hp], qA[:, hp],
                                     start=first_g, stop=last_g,
                                     perf_mode=DR, skip_group_check=True)
                    nc.tensor.matmul(gram2_ps[0:64, 128:192], qB[:, hp], qB[:, hp],
                                     start=False, stop=last_g,
                                     perf_mode=DR, skip_group_check=True)
                    nc.tensor.matmul(gram2_ps[0:128, 192:320], kA[:, hp], kA[:, hp],
                                     start=False, stop=last_g,
                                     perf_mode=DR, skip_group_check=True)
                    nc.tensor.matmul(gram2_ps[0:64, 320:384], kB[:, hp], kB[:, hp],
                                     start=False, stop=last_g,
                                     perf_mode=DR, skip_group_check=True)

            dw_v_tail()

        gram_sb = consts.tile([128, 384], F32, tag="gsb")
        gram2_sb = consts.tile([128, 384], F32, tag="g2sb")
        nc.vector.memset(gram_sb[64:128, 192:384], 0.0)

        nc.vector.tensor_copy(out=gram_sb[:, 0:192], in_=gram_ps[0:128, 0:192])
        nc.vector.tensor_copy(out=gram_sb[0:64, 192:384],
                              in_=gram_ps[0:64, 192:384])
        nc.vector.tensor_copy(out=gram2_sb, in_=gram2_ps[0:128, 0:384])
        nc.sync.dma_start(out=gram_out, in_=gram_sb)
        nc.sync.dma_start(out=sumsq_out, in_=gram2_sb)
    nc.compile()
    return nc


def _build_phase2():
    nc = bacc.Bacc("TRN2", target_bir_lowering=False, debug=False,
                   num_devices=N_CORES)
    v_in = nc.dram_tensor("v_sp", [C, ROWS * W], F16, kind="ExternalInput").ap()
    mwT = nc.dram_tensor("mwT", [C, C], F16, kind="ExternalInput").ap()
    out_loc = nc.dram_tensor("out_loc", [C, ROWS * W], F16, kind="ExternalOutput").ap()

    BT = 4096
    NT = ROWS * W // BT
    SUB = 512
    with ExitStack() as ctx:
        tc = ctx.enter_context(tile.TileContext(nc))
        consts = ctx.enter_context(tc.tile_pool(name="consts", bufs=1))
        vpool = ctx.enter_context(tc.tile_pool(name="vpool", bufs=6))
        aopool = ctx.enter_context(tc.tile_pool(name="aopool", bufs=6))
        ps_pj = ctx.enter_context(tc.tile_pool(name="ps_pj", bufs=3, space="PSUM"))

        mw = consts.tile([96, 2, C], F16, tag="mw")
        nc.sync.dma_start(out=mw[:, 0, :], in_=mwT[0:96, :])
        nc.sync.dma_start(out=mw[:, 1, :], in_=mwT[96:192, :])

        for t in range(NT):
            px = slice(t * BT, (t + 1) * BT)
            va = vpool.tile([96, BT], F16, tag="va")
            vb = vpool.tile([96, BT], F16, tag="vb")
            nc.sync.dma_start(out=va, in_=v_in[0:96, px])
            nc.sync.dma_start(out=vb, in_=v_in[96:192, px])
            oja = aopool.tile([128, BT], F16, tag="oja")
            ojb = aopool.tile([64, BT], F16, tag="ojb")
            for h in range(BT // SUB):
                hs = slice(h * SUB, (h + 1) * SUB)
                pja = ps_pj.tile([128, SUB], F32, tag="pja")
                pjb = ps_pj.tile([64, SUB], F32, tag="pjb")
                nc.tensor.matmul(pja, mw[:, 0, 0:128], va[:, hs],
                                 start=True, stop=False)
                nc.tensor.matmul(pja, mw[:, 1, 0:128], vb[:, hs],
                                 start=False, stop=True)
                nc.tensor.matmul(pjb, mw[:, 0, 128:192], va[:, hs],
                                 start=True, stop=False)
                nc.tensor.matmul(pjb, mw[:, 1, 128:192], vb[:, hs],
                                 start=False, stop=True)
                nc.scalar.copy(out=oja[:, hs], in_=pja)
                nc.vector.tensor_copy(out=ojb[:, hs], in_=pjb)
            nc.sync.dma_start(out=out_loc[0:128, px], in_=oja)
            nc.sync.dma_start(out=out_loc[128:192, px], in_=ojb)
    nc.compile()
    return nc


_NC1 = None
_NC2 = None
_LAST_R1 = None
_LAST_R2 = None


def _get_programs():
    global _NC1, _NC2
    if _NC1 is None:
        _NC1 = _build_phase1()
        _NC2 = _build_phase2()
    return _NC1, _NC2


def _q8(a):
    return a.astype(NP8)


def kernel(x, qkv_w, dw_w, proj_w, temperature, _trace=False):
    x = np.asarray(x, dtype=np.float32)
    qkv_w = np.asarray(qkv_w, dtype=np.float32)
    dw_w = np.asarray(dw_w, dtype=np.float32)
    proj_w = np.asarray(proj_w, dtype=np.float32)
    temperature = np.asarray(temperature, dtype=np.float32)

    nc1, nc2 = _get_programs()

    # ---- weight prep ----
    qkv_wT = np.ascontiguousarray(qkv_w[:, :, 0, 0].T)     # [192 in, 576 out]
    wT_pad = np.zeros((256, C3), np.float32)
    wT_pad[0:192] = qkv_wT
    w8 = _q8(wT_pad).astype(np.float32)
    w_lo = _q8(wT_pad - w8)
    w_hi = _q8(wT_pad)
    wqk_np = np.zeros((128, 3, 2, 128), NP8)
    for ci in range(3):
        for kt in range(2):
            wqk_np[:, ci, kt] = w_hi[kt * 128:(kt + 1) * 128,
                                     ci * 128:(ci + 1) * 128]
    wv_np = np.zeros((128, 2, 2, 2, 128), NP8)
    for cj, (c0, cw) in enumerate(((384, 128), (512, 64))):
        for kt in range(2):
            wv_np[:, cj, 0, kt, 0:cw] = w_hi[kt * 128:(kt + 1) * 128, c0:c0 + cw]
            wv_np[:, cj, 1, kt, 0:cw] = w_lo[kt * 128:(kt + 1) * 128, c0:c0 + cw]

    dw_flat = np.ascontiguousarray(dw_w[:, 0].reshape(C3, 9))
    d8 = _q8(dw_flat).astype(np.float32)
    d_hi = _q8(dw_flat)
    d_lo = _q8(dw_flat - d8)
    dq_np = np.zeros((128, 3, 5, 2, 128), NP8)
    for ci in range(3):
        for pi, (tA, tB) in enumerate(TAP_PAIRS):
            dA = np.diag(d_hi[ci * 128:(ci + 1) * 128, tA].astype(np.float32))
            dq_np[:, ci, pi, 0] = dA.astype(NP8)
            if tB != tA:
                dBv = np.diag(d_hi[ci * 128:(ci + 1) * 128, tB].astype(np.float32))
                dq_np[:, ci, pi, 1] = dBv.astype(NP8)
    dv_np = np.zeros((128, 2, 5, 2, 128), NP8)
    for ws, dsrc in ((0, d_hi), (1, d_lo)):
        for pi, (tA, tB) in enumerate(TAP_PAIRS):
            dA = np.diag(dsrc[384:512, tA].astype(np.float32))
            dv_np[:, ws, pi, 0] = dA.astype(NP8)
            if tB != tA:
                dBv = np.diag(dsrc[384:512, tB].astype(np.float32))
                dv_np[:, ws, pi, 1] = dBv.astype(NP8)
    dv4_np = np.zeros((128, 2, 5, 2, 64), NP8)
    dh4 = d_hi[512:576].astype(np.float32)
    dl4 = d_lo[512:576].astype(np.float32)
    for pi, (tA, tB) in enumerate(TAP_PAIRS):
        for kt, t in ((0, tA), (1, tB)):
            if kt == 1 and tB == tA:
                continue
            dv4_np[0:64, 0, pi, kt] = np.diag(dh4[:, t]).astype(NP8)
            dv4_np[64:128, 0, pi, kt] = np.diag(dh4[:, t]).astype(NP8)
            dv4_np[0:64, 1, pi, kt] = np.diag(dl4[:, t]).astype(NP8)
    ident_np = np.eye(128, dtype=np.float16)

    # ---- x prep: hi/lo fp8, kt-padded, halo rows ----
    in_maps1 = []
    for core in range(N_CORES):
        b, half = divmod(core, 2)
        base = half * ROWS
        x_pad = np.zeros((256, ROWS + 2, W), np.float32)
        lo, hi = base - 1, base + ROWS + 1
        slo, shi = max(lo, 0), min(hi, H)
        x_pad[0:192, slo - lo:shi - lo, :] = x[b, :, slo:shi, :]
        xh = _q8(x_pad)
        xl = _q8(x_pad - xh.astype(np.float32))
        x_hl = np.stack([xh, xl], axis=0)
        in_maps1.append({"x_hl": x_hl, "wqk": wqk_np, "wv": wv_np,
                         "dq": dq_np, "dv": dv_np, "dv4": dv4_np,
                         "ident_in": ident_np})

    global _LAST_R1, _LAST_R2
    r1 = run_bass_kernel_spmd(nc1, in_maps1, core_ids=list(range(N_CORES)),
                              trace=_trace)
    _LAST_R1 = r1

    # ---- host: combine partials, softmax, fold proj into attn ----
    proj_w2 = proj_w[:, :, 0, 0].astype(np.float64)
    mwTs = np.zeros((B, C, C), np.float16)
    for b in range(B):
        ra, rb = r1.results[2 * b], r1.results[2 * b + 1]
        gp = ra["gram_part"].astype(np.float64) + rb["gram_part"].astype(np.float64)
        G = np.zeros((192, 192))
        G[0:128, 0:128] = gp[0:128, 0:128]
        G[0:128, 128:192] = gp[0:128, 128:192]
        G[128:192, 0:128] = gp[0:64, 192:320]
        G[128:192, 128:192] = gp[0:64, 320:384]
        g2 = ra["sumsq_part"].astype(np.float64) + rb["sumsq_part"].astype(np.float64)
        qsq = np.concatenate([np.diag(g2[0:128, 0:128]),
                              np.diag(g2[0:64, 128:192])])
        ksq = np.concatenate([np.diag(g2[0:128, 192:320]),
                              np.diag(g2[0:64, 320:384])])
        qn = np.maximum(np.sqrt(qsq), EPS)
        kn = np.maximum(np.sqrt(ksq), EPS)
        attn_bd = np.zeros((C, C))
        for h in range(HEADS):
            sl = slice(h * DH, (h + 1) * DH)
            Gh = G[sl, sl] / np.outer(qn[sl], kn[sl]) * float(temperature[h, 0, 0])
            Gh = Gh - Gh.max(axis=1, keepdims=True)
            e = np.exp(Gh)
            attn_bd[sl, sl] = e / e.sum(axis=1, keepdims=True)
        mwTs[b] = (proj_w2 @ attn_bd).T.astype(np.float16)

    in_maps2 = []
    for core in range(N_CORES):
        b = core // 2
        in_maps2.append({"v_sp": r1.results[core]["v_sp"], "mwT": mwTs[b]})
    r2 = run_bass_kernel_spmd(nc2, in_maps2, core_ids=list(range(N_CORES)),
                              trace=_trace)
    _LAST_R2 = r2

    out = np.zeros((B, C, H, W), np.float32)
    for core in range(N_CORES):
        b, half = divmod(core, 2)
        out[b, :, half * ROWS:(half + 1) * ROWS, :] = \
            r2.results[core]["out_loc"].reshape(C, ROWS, W).astype(np.float32)
    return out


# revision 5
# speedup vs baseline: 1.0783x; 1.0209x over previous
"""Trainium2 Bass kernel for nn_Attention_4415226380830 (XCA channel attention),
restructured around fp8 DoubleRow matmuls.

Dtype strategy (validated vs reference in numpy):
  - q,k path: fully fp8e4m3 (errors wash out in the 65536-px Gram).
  - v path: x and conv weights split hi+lo fp8 (3-term products ~= fp16
    accuracy at 2x fp16 matmul rate); dw for v-chunk3 likewise hi/lo DR
    pairs on PE; v-chunk4 (64 ch) taps on DVE/Pool at fp16.
  - v spill + out: fp16.

PE work per macro (16 rows): conv qk 3xDR + v 2x3xDR, dw as DR *tap pairs*
(2 shifted windows per matmul via a stride-delta kt dim), packed fp16-view
transposes (2 px/row), DR gram (256 px per pass). Norms via ACT Square+accum
on fp8 acc. Evacs split ACT/DVE (Pool cannot read PSUM).
"""

import numpy as np
from contextlib import ExitStack

import ml_dtypes
import concourse.bass as bass
from concourse import bacc
import concourse.mybir as mybir
import concourse.tile as tile
from concourse.bass_utils import run_bass_kernel_spmd

F32 = mybir.dt.float32
F16 = mybir.dt.float16
F8 = mybir.dt.float8e4
NP8 = ml_dtypes.float8_e4m3fn
DR = mybir.MatmulPerfMode.DoubleRow

B = 4
C = 192
HEADS = 4
DH = C // HEADS
H = 256
W = 256
C3 = 3 * C
N_CORES = 8
EPS = 1e-12

ROWS = H // 2            # rows per core
MROWS = 16               # rows per macro-tile
NMACRO = ROWS // MROWS   # 8
WIN_ROWS = MROWS + 2     # 18
PXM = MROWS * W          # 4096

# conv psum substeps over the 18-row window: 4+4+4+4+2 rows
CONV_STEPS = [(r, min(4, WIN_ROWS - r)) for r in range(0, WIN_ROWS, 4)]
# dw substeps: 4 rows = 1024 px each
DWSTEPS = 4
DWROWS = MROWS // DWSTEPS  # 4 rows per substep
# tap pairs (t = 3*(dy+1) + (dx+1)); last pair duplicates tap 8 with zero B
TAP_PAIRS = [(0, 1), (2, 3), (4, 5), (6, 7), (8, 8)]


def _tap_delta(tA, tB):
    dyA, dxA = tA // 3 - 1, tA % 3 - 1
    dyB, dxB = tB // 3 - 1, tB % 3 - 1
    return (dyB - dyA) * 258 + (dxB - dxA)


def _dw_rhs(win, cp, si2, tA, tB):
    """DR moving AP: pair of shifted 2-row windows from win [cp,18,258]."""
    dyA, dxA = tA // 3 - 1, tA % 3 - 1
    base = win[0:cp, 1 + dyA + 2 * si2:1 + dyA + 2 * si2 + 2,
               1 + dxA:257 + dxA]
    return bass.AP(tensor=win.tensor, offset=base.offset,
                   ap=[[win.ap[0][0], cp], [_tap_delta(tA, tB), 2],
                       [258, 2], [1, 256]])


def _build_phase1():
    nc = bacc.Bacc("TRN2", target_bir_lowering=False, debug=False,
                   num_devices=N_CORES)
    # x hi/lo, kt-padded to 256 channels (192:256 zeroed on host)
    x_hl = nc.dram_tensor("x_hl", [2, 256, ROWS + 2, W], F8,
                          kind="ExternalInput").ap()
    wqk_in = nc.dram_tensor("wqk", [128, 3, 2, 128], F8, kind="ExternalInput").ap()
    wv_in = nc.dram_tensor("wv", [128, 2, 2, 2, 128], F8, kind="ExternalInput").ap()
    dq_in = nc.dram_tensor("dq", [128, 3, 5, 2, 128], F8, kind="ExternalInput").ap()
    dv_in = nc.dram_tensor("dv", [128, 2, 6, 2, 128], F8, kind="ExternalInput").ap()
    dv4_in = nc.dram_tensor("dv4", [128, 2, 5, 2, 64], F8, kind="ExternalInput").ap()
    ident_in = nc.dram_tensor("ident_in", [128, 128], F16, kind="ExternalInput").ap()

    gram_out = nc.dram_tensor("gram_part", [128, 384], F32, kind="ExternalOutput").ap()
    sumsq_out = nc.dram_tensor("sumsq_part", [128, 384], F32,
                               kind="ExternalOutput").ap()
    v_out = nc.dram_tensor("v_sp", [C, ROWS * W], F16, kind="ExternalOutput").ap()

    with ExitStack() as ctx:
        tc = ctx.enter_context(tile.TileContext(nc))
        consts = ctx.enter_context(tc.tile_pool(name="consts", bufs=1))
        xpool = ctx.enter_context(tc.tile_pool(name="xpool", bufs=3))
        winp = ctx.enter_context(tc.tile_pool(name="winp", bufs=4))
        winvp = ctx.enter_context(tc.tile_pool(name="winvp", bufs=3))
        accp = ctx.enter_context(tc.tile_pool(name="accp", bufs=5))
        vaccp = ctx.enter_context(tc.tile_pool(name="vaccp", bufs=3))
        qkTp = ctx.enter_context(tc.tile_pool(name="qkTp", bufs=4))
        ps_big = ctx.enter_context(tc.tile_pool(name="ps_big", bufs=4, space="PSUM"))
        ps_t = ctx.enter_context(tc.tile_pool(name="ps_t", bufs=2, space="PSUM"))
        ps_g = ctx.enter_context(tc.tile_pool(name="ps_g", bufs=1, space="PSUM"))
        ps_g2 = ctx.enter_context(tc.tile_pool(name="ps_g2", bufs=1, space="PSUM"))

        wqk = consts.tile([128, 3, 2, 128], F8, tag="wqk")
        wv = consts.tile([128, 2, 2, 2, 128], F8, tag="wv")
        dq = consts.tile([128, 3, 5, 2, 128], F8, tag="dq")
        dv = consts.tile([128, 2, 6, 2, 128], F8, tag="dv")
        dv4 = consts.tile([128, 2, 5, 2, 64], F8, tag="dv4")
        ident = consts.tile([128, 128], F16, tag="ident")
        nc.sync.dma_start(out=wqk, in_=wqk_in)
        nc.sync.dma_start(out=wv, in_=wv_in)
        nc.sync.dma_start(out=dq, in_=dq_in)
        nc.sync.dma_start(out=dv, in_=dv_in)
        nc.sync.dma_start(out=dv4, in_=dv4_in)
        nc.sync.dma_start(out=ident, in_=ident_in)

        gram_ps = ps_g.tile([128, 384], F32)
        gram2_ps = ps_g2.tile([128, 384], F32)

        for mj in range(NMACRO):
            r0 = MROWS * mj
            # x tile [128, kt, hl, 18, 256]
            xt = xpool.tile([128, 2, 2, WIN_ROWS, W], F8, tag="xt")
            for kt in range(2):
                for hl in range(2):
                    nc.sync.dma_start(
                        out=xt[:, kt, hl],
                        in_=x_hl[hl, 128 * kt:128 * (kt + 1), r0:r0 + WIN_ROWS, :])

            accs = {}
            wins = {}

            def conv_qk(ci):
                win = winp.tile([128, WIN_ROWS, 258], F8, tag="win")
                nc.gpsimd.memset(win[:, :, 0:258:257], 0.0)
                for st in range(WIN_ROWS // 2):
                    sr = 2 * st
                    pc = ps_big.tile([128, 512], F32, tag="pw")
                    nc.tensor.matmul(pc, wqk[:, ci],
                                     xt[:, :, 0, sr:sr + 2, :],
                                     start=True, stop=True, perf_mode=DR)
                    dst = win[:, sr:sr + 2, 1:257]
                    srcc = pc.rearrange("p (r w) -> p r w", w=W)
                    if st % 2 == 0:
                        nc.scalar.copy(out=dst, in_=srcc)
                    else:
                        nc.vector.tensor_copy(out=dst, in_=srcc)
                wins[ci] = win

            def conv_v(cj):
                cp = 128 if cj == 0 else 64
                if cj == 0:
                    whl3 = winvp.tile([128, 2, WIN_ROWS, 258], F8, tag="whl0")
                    wh, wl = whl3[:, 0], whl3[:, 1]
                else:
                    whl = winvp.tile([128, WIN_ROWS, 258], F8, tag="whl1")
                    wh, wl = whl[0:64], whl[64:128]
                nc.gpsimd.memset(wh[:, :, 0:258:257], 0.0)
                nc.gpsimd.memset(wl[:, :, 0:258:257], 0.0)
                for st in range(WIN_ROWS // 2):
                    sr = 2 * st
                    rs = slice(sr, sr + 2)
                    pc = ps_big.tile([128, 512], F32, tag="pw")
                    nc.tensor.matmul(pc[0:cp], wv[:, cj, 0, :, 0:cp],
                                     xt[:, :, 0, rs, :],
                                     start=True, stop=False, perf_mode=DR)
                    nc.tensor.matmul(pc[0:cp], wv[:, cj, 0, :, 0:cp],
                                     xt[:, :, 1, rs, :],
                                     start=False, stop=False, perf_mode=DR)
                    nc.tensor.matmul(pc[0:cp], wv[:, cj, 1, :, 0:cp],
                                     xt[:, :, 0, rs, :],
                                     start=False, stop=True, perf_mode=DR)
                    srcv = pc[0:cp].rearrange("p (r w) -> p r w", w=W)
                    nc.scalar.copy(out=wh[:, rs, 1:257], in_=srcv)
                    nc.vector.tensor_tensor(out=wl[:, rs, 1:257],
                                            in0=srcv, in1=wh[:, rs, 1:257],
                                            op=mybir.AluOpType.subtract)
                wins[3 + cj] = (wh, wl, cp) if cj == 0 else (whl, wl, cp)

            def dw_qk(ci):
                acc = accp.tile([128, PXM], F8, tag="acc")
                for si2 in range(2 * DWSTEPS):
                    pd = ps_big.tile([128, 512], F32, tag="pw")
                    for pi, (tA, tB) in enumerate(TAP_PAIRS):
                        nc.tensor.matmul(pd, dq[:, ci, pi],
                                         _dw_rhs(wins[ci], 128, si2, tA, tB),
                                         start=(pi == 0), stop=(pi == 4),
                                         perf_mode=DR)
                    if si2 % 2 == 0:
                        nc.scalar.copy(out=acc[:, si2 * 512:(si2 + 1) * 512], in_=pd)
                    else:
                        nc.vector.tensor_copy(out=acc[:, si2 * 512:(si2 + 1) * 512],
                                              in_=pd)
                accs[ci] = acc

            def dw_v(cj, vacc):
                wh, wl, cp = wins[3 + cj]
                if cj == 0:
                    plan = ([(dv, 0, pi, wh) for pi in range(5)]
                            + [(dv, 0, pi, wl) for pi in range(4)]
                            + [(dv, 1, pi, wh) for pi in range(4)])
                else:
                    plan = [(dv4, 0, pi, wh) for pi in range(5)] \
                        + [(dv4, 1, pi, wh) for pi in range(5)]
                for si2 in range(2 * DWSTEPS):
                    pd = ps_big.tile([128, 512], F32, tag="pw")
                    for k, (dvt, wset, pi, wn) in enumerate(plan):
                        nc.tensor.matmul(pd[0:cp],
                                         dvt[:, wset, pi],
                                         _dw_rhs(wn, 128, si2, *TAP_PAIRS[pi]),
                                         start=(k == 0),
                                         stop=(cj == 1 and k == len(plan) - 1),
                                         perf_mode=DR)
                    if cj == 0:
                        # shared pass: kt0 = wl8 x h[t8], kt1 = wh8 x l[t8]
                        # (in-tile kt stride = +WIN_ROWS*258)
                        base = wh[0:128, 1 + 1 + 2 * si2:3 + 1 + 2 * si2, 2:258]
                        rhs = bass.AP(tensor=wh.tensor, offset=base.offset,
                                      ap=[[wh.ap[0][0], 128],
                                          [WIN_ROWS * 258, 2], [258, 2], [1, 256]])
                        nc.tensor.matmul(pd[0:cp], dv[:, 0, 5], rhs,
                                         start=False, stop=True, perf_mode=DR)
                    if si2 % 2 == 0:
                        nc.scalar.copy(out=vacc[:, si2 * 512:(si2 + 1) * 512],
                                       in_=pd[0:cp])
                    else:
                        nc.vector.tensor_copy(out=vacc[:, si2 * 512:(si2 + 1) * 512],
                                              in_=pd[0:cp])

            vacc3 = vaccp.tile([128, PXM], F16, tag="v3")
            vacc4 = vaccp.tile([64, PXM], F16, tag="v4")
            # software pipeline: conv(i+1) issues before dw(i)
            conv_qk(0)
            conv_qk(1)
            dw_qk(0)
            conv_qk(2)
            dw_qk(1)
            conv_v(0)
            dw_qk(2)
            conv_v(1)

            def dw_v_tail():
                dw_v(0, vacc3)
                dw_v(1, vacc4)
                nc.sync.dma_start(out=v_out[0:128, mj * PXM:(mj + 1) * PXM],
                                  in_=vacc3)
                nc.sync.dma_start(out=v_out[128:192, mj * PXM:(mj + 1) * PXM],
                                  in_=vacc4)

            # ======== Phase C: transposes/grams overlap dw_v ========
            a16 = [accs[ci].bitcast(F16) for ci in range(3)]   # [128, 2048]
            for pg in range(PXM // 512):    # 2 pr per group
                tps = ps_t.tile([128, 2, 384], F16, tag="tps")
                for hp in range(2):
                    for ci in range(3):
                        nc.tensor.matmul(
                            tps[:, hp, ci * 128:(ci + 1) * 128],
                            a16[ci][:, pg * 256 + hp * 128:pg * 256 + hp * 128 + 128],
                            ident, is_transpose=True,
                            start=(hp == 0 and ci == 0), stop=(hp == 1 and ci == 2),
                            skip_group_check=True)
                t8 = tps.bitcast(F8)  # [128, 2, 768]
                qA = qkTp.tile([128, 2, 2, 128], F8, tag="qA")
                qB = qkTp.tile([128, 2, 2, 64], F8, tag="qB")
                kA = qkTp.tile([128, 2, 2, 128], F8, tag="kA")
                kB = qkTp.tile([128, 2, 2, 64], F8, tag="kB")

                def deint(off, n):
                    return bass.AP(tensor=t8.tensor, offset=t8.offset + off,
                                   ap=[[t8.ap[0][0], 128], [768, 2], [1, 2], [2, n]])
                nc.scalar.copy(out=qA, in_=deint(0, 128))
                nc.scalar.copy(out=qB, in_=deint(256, 64))
                nc.vector.tensor_copy(out=kA, in_=deint(384, 128))
                nc.vector.tensor_copy(out=kB, in_=deint(640, 64))

                for hp in range(2):
                    first_g = (mj == 0 and pg == 0 and hp == 0)
                    last_g = (mj == NMACRO - 1 and pg == PXM // 512 - 1 and hp == 1)
                    nc.tensor.matmul(gram_ps[0:128, 0:128], qA[:, hp], kA[:, hp],
                                     start=first_g, stop=last_g,
                                     perf_mode=DR, skip_group_check=True)
                    nc.tensor.matmul(gram_ps[0:128, 128:192], qA[:, hp], kB[:, hp],
                                     start=False, stop=last_g,
                                     perf_mode=DR, skip_group_check=True)
                    nc.tensor.matmul(gram_ps[0:64, 192:320], qB[:, hp], kA[:, hp],
                                     start=False, stop=last_g,
                                     perf_mode=DR, skip_group_check=True)
                    nc.tensor.matmul(gram_ps[0:64, 320:384], qB[:, hp], kB[:, hp],
                                     start=False, stop=last_g,
                                     perf_mode=DR, skip_group_check=True)
                    nc.tensor.matmul(gram2_ps[0:128, 0:128], qA[:, hp], qA[:, hp],
                                     start=first_g, stop=last_g,
                                     perf_mode=DR, skip_group_check=True)
                    nc.tensor.matmul(gram2_ps[0:64, 128:192], qB[:, hp], qB[:, hp],
                                     start=False, stop=last_g,
                                     perf_mode=DR, skip_group_check=True)
                    nc.tensor.matmul(gram2_ps[0:128, 192:320], kA[:, hp], kA[:, hp],
                                     start=False, stop=last_g,
                                     perf_mode=DR, skip_group_check=True)
                    nc.tensor.matmul(gram2_ps[0:64, 320:384], kB[:, hp], kB[:, hp],
                                     start=False, stop=last_g,
                                     perf_mode=DR, skip_group_check=True)

            dw_v_tail()

        gram_sb = consts.tile([128, 384], F32, tag="gsb")
        gram2_sb = consts.tile([128, 384], F32, tag="g2sb")
        nc.vector.memset(gram_sb[64:128, 192:384], 0.0)

        nc.vector.tensor_copy(out=gram_sb[:, 0:192], in_=gram_ps[0:128, 0:192])
        nc.vector.tensor_copy(out=gram_sb[0:64, 192:384],
                              in_=gram_ps[0:64, 192:384])
        nc.vector.tensor_copy(out=gram2_sb, in_=gram2_ps[0:128, 0:384])
        nc.sync.dma_start(out=gram_out, in_=gram_sb)
        nc.sync.dma_start(out=sumsq_out, in_=gram2_sb)
    nc.compile()
    return nc


def _build_phase2():
    nc = bacc.Bacc("TRN2", target_bir_lowering=False, debug=False,
                   num_devices=N_CORES)
    v_in = nc.dram_tensor("v_sp", [C, ROWS * W], F16, kind="ExternalInput").ap()
    mwT = nc.dram_tensor("mwT", [C, C], F16, kind="ExternalInput").ap()
    out_loc = nc.dram_tensor("out_loc", [C, ROWS * W], F16, kind="ExternalOutput").ap()

    BT = 4096
    NT = ROWS * W // BT
    SUB = 512
    with ExitStack() as ctx:
        tc = ctx.enter_context(tile.TileContext(nc))
        consts = ctx.enter_context(tc.tile_pool(name="consts", bufs=1))
        vpool = ctx.enter_context(tc.tile_pool(name="vpool", bufs=6))
        aopool = ctx.enter_context(tc.tile_pool(name="aopool", bufs=6))
        ps_pj = ctx.enter_context(tc.tile_pool(name="ps_pj", bufs=3, space="PSUM"))

        mw = consts.tile([96, 2, C], F16, tag="mw")
        nc.sync.dma_start(out=mw[:, 0, :], in_=mwT[0:96, :])
        nc.sync.dma_start(out=mw[:, 1, :], in_=mwT[96:192, :])

        for t in range(NT):
            px = slice(t * BT, (t + 1) * BT)
            va = vpool.tile([96, BT], F16, tag="va")
            vb = vpool.tile([96, BT], F16, tag="vb")
            nc.sync.dma_start(out=va, in_=v_in[0:96, px])
            nc.sync.dma_start(out=vb, in_=v_in[96:192, px])
            oja = aopool.tile([128, BT], F16, tag="oja")
            ojb = aopool.tile([64, BT], F16, tag="ojb")
            for h in range(BT // SUB):
                hs = slice(h * SUB, (h + 1) * SUB)
                pja = ps_pj.tile([128, SUB], F32, tag="pja")
                pjb = ps_pj.tile([64, SUB], F32, tag="pjb")
                nc.tensor.matmul(pja, mw[:, 0, 0:128], va[:, hs],
                                 start=True, stop=False)
                nc.tensor.matmul(pja, mw[:, 1, 0:128], vb[:, hs],
                                 start=False, stop=True)
                nc.tensor.matmul(pjb, mw[:, 0, 128:192], va[:, hs],
                                 start=True, stop=False)
                nc.tensor.matmul(pjb, mw[:, 1, 128:192], vb[:, hs],
                                 start=False, stop=True)
                nc.scalar.copy(out=oja[:, hs], in_=pja)
                nc.vector.tensor_copy(out=ojb[:, hs], in_=pjb)
            nc.sync.dma_start(out=out_loc[0:128, px], in_=oja)
            nc.sync.dma_start(out=out_loc[128:192, px], in_=ojb)
    nc.compile()
    return nc


_NC1 = None
_NC2 = None
_LAST_R1 = None
_LAST_R2 = None


def _get_programs():
    global _NC1, _NC2
    if _NC1 is None:
        _NC1 = _build_phase1()
        _NC2 = _build_phase2()
    return _NC1, _NC2


def _q8(a):
    return a.astype(NP8)


def kernel(x, qkv_w, dw_w, proj_w, temperature, _trace=False):
    x = np.asarray(x, dtype=np.float32)
    qkv_w = np.asarray(qkv_w, dtype=np.float32)
    dw_w = np.asarray(dw_w, dtype=np.float32)
    proj_w = np.asarray(proj_w, dtype=np.float32)
    temperature = np.asarray(temperature, dtype=np.float32)

    nc1, nc2 = _get_programs()

    # ---- weight prep ----
    qkv_wT = np.ascontiguousarray(qkv_w[:, :, 0, 0].T)     # [192 in, 576 out]
    wT_pad = np.zeros((256, C3), np.float32)
    wT_pad[0:192] = qkv_wT
    w8 = _q8(wT_pad).astype(np.float32)
    w_lo = _q8(wT_pad - w8)
    w_hi = _q8(wT_pad)
    wqk_np = np.zeros((128, 3, 2, 128), NP8)
    for ci in range(3):
        for kt in range(2):
            wqk_np[:, ci, kt] = w_hi[kt * 128:(kt + 1) * 128,
                                     ci * 128:(ci + 1) * 128]
    wv_np = np.zeros((128, 2, 2, 2, 128), NP8)
    for cj, (c0, cw) in enumerate(((384, 128), (512, 64))):
        for kt in range(2):
            wv_np[:, cj, 0, kt, 0:cw] = w_hi[kt * 128:(kt + 1) * 128, c0:c0 + cw]
            wv_np[:, cj, 1, kt, 0:cw] = w_lo[kt * 128:(kt + 1) * 128, c0:c0 + cw]

    dw_flat = np.ascontiguousarray(dw_w[:, 0].reshape(C3, 9))
    d8 = _q8(dw_flat).astype(np.float32)
    d_hi = _q8(dw_flat)
    d_lo = _q8(dw_flat - d8)
    dq_np = np.zeros((128, 3, 5, 2, 128), NP8)
    for ci in range(3):
        for pi, (tA, tB) in enumerate(TAP_PAIRS):
            dA = np.diag(d_hi[ci * 128:(ci + 1) * 128, tA].astype(np.float32))
            dq_np[:, ci, pi, 0] = dA.astype(NP8)
            if tB != tA:
                dBv = np.diag(d_hi[ci * 128:(ci + 1) * 128, tB].astype(np.float32))
                dq_np[:, ci, pi, 1] = dBv.astype(NP8)
    dv_np = np.zeros((128, 2, 6, 2, 128), NP8)
    for ws, dsrc in ((0, d_hi), (1, d_lo)):
        for pi, (tA, tB) in enumerate(TAP_PAIRS):
            dA = np.diag(dsrc[384:512, tA].astype(np.float32))
            dv_np[:, ws, pi, 0] = dA.astype(NP8)
            if tB != tA:
                dBv = np.diag(dsrc[384:512, tB].astype(np.float32))
                dv_np[:, ws, pi, 1] = dBv.astype(NP8)
    dv_np[:, 0, 5, 0] = np.diag(d_lo[384:512, 8].astype(np.float32)).astype(NP8)
    dv_np[:, 0, 5, 1] = np.diag(d_hi[384:512, 8].astype(np.float32)).astype(NP8)
    dv4_np = np.zeros((128, 2, 5, 2, 64), NP8)
    dh4 = d_hi[512:576].astype(np.float32)
    dl4 = d_lo[512:576].astype(np.float32)
    for pi, (tA, tB) in enumerate(TAP_PAIRS):
        for kt, t in ((0, tA), (1, tB)):
            if kt == 1 and tB == tA:
                continue
            dv4_np[0:64, 0, pi, kt] = np.diag(dh4[:, t]).astype(NP8)
            dv4_np[64:128, 0, pi, kt] = np.diag(dh4[:, t]).astype(NP8)
            dv4_np[0:64, 1, pi, kt] = np.diag(dl4[:, t]).astype(NP8)
    ident_np = np.eye(128, dtype=np.float16)

    # ---- x prep: hi/lo fp8, kt-padded, halo rows ----
    in_maps1 = []
    for core in range(N_CORES):
        b, half = divmod(core, 2)
        base = half * ROWS
        x_pad = np.zeros((256, ROWS + 2, W), np.float32)
        lo, hi = base - 1, base + ROWS + 1
        slo, shi = max(lo, 0), min(hi, H)
        x_pad[0:192, slo - lo:shi - lo, :] = x[b, :, slo:shi, :]
        xh = _q8(x_pad)
        xl = _q8(x_pad - xh.astype(np.float32))
        x_hl = np.stack([xh, xl], axis=0)
        in_maps1.append({"x_hl": x_hl, "wqk": wqk_np, "wv": wv_np,
                         "dq": dq_np, "dv": dv_np, "dv4": dv4_np,
                         "ident_in": ident_np})

    global _LAST_R1, _LAST_R2
    r1 = run_bass_kernel_spmd(nc1, in_maps1, core_ids=list(range(N_CORES)),
                              trace=_trace)
    _LAST_R1 = r1

    # ---- host: combine partials, softmax, fold proj into attn ----
    proj_w2 = proj_w[:, :, 0, 0].astype(np.float64)
    mwTs = np.zeros((B, C, C), np.float16)
    for b in range(B):
        ra, rb = r1.results[2 * b], r1.results[2 * b + 1]
        gp = ra["gram_part"].astype(np.float64) + rb["gram_part"].astype(np.float64)
        G = np.zeros((192, 192))
        G[0:128, 0:128] = gp[0:128, 0:128]
        G[0:128, 128:192] = gp[0:128, 128:192]
        G[128:192, 0:128] = gp[0:64, 192:320]
        G[128:192, 128:192] = gp[0:64, 320:384]
        g2 = ra["sumsq_part"].astype(np.float64) + rb["sumsq_part"].astype(np.float64)
        qsq = np.concatenate([np.diag(g2[0:128, 0:128]),
                              np.diag(g2[0:64, 128:192])])
        ksq = np.concatenate([np.diag(g2[0:128, 192:320]),
                              np.diag(g2[0:64, 320:384])])
        qn = np.maximum(np.sqrt(qsq), EPS)
        kn = np.maximum(np.sqrt(ksq), EPS)
        attn_bd = np.zeros((C, C))
        for h in range(HEADS):
            sl = slice(h * DH, (h + 1) * DH)
            Gh = G[sl, sl] / np.outer(qn[sl], kn[sl]) * float(temperature[h, 0, 0])
            Gh = Gh - Gh.max(axis=1, keepdims=True)
            e = np.exp(Gh)
            attn_bd[sl, sl] = e / e.sum(axis=1, keepdims=True)
        mwTs[b] = (proj_w2 @ attn_bd).T.astype(np.float16)

    in_maps2 = []
    for core in range(N_CORES):
        b = core // 2
        in_maps2.append({"v_sp": r1.results[core]["v_sp"], "mwT": mwTs[b]})
    r2 = run_bass_kernel_spmd(nc2, in_maps2, core_ids=list(range(N_CORES)),
                              trace=_trace)
    _LAST_R2 = r2

    out = np.zeros((B, C, H, W), np.float32)
    for core in range(N_CORES):
        b, half = divmod(core, 2)
        out[b, :, half * ROWS:(half + 1) * ROWS, :] = \
            r2.results[core]["out_loc"].reshape(C, ROWS, W).astype(np.float32)
    return out
